# revision 21
# baseline (speedup 1.0000x reference)
"""Causal multi-head self-attention (RoPE) Trainium2 Bass kernel.

Problem: x:(4,2048,1024), Wq/Wk/Wv:(1024,1024), Wo:(1024,1024), bo:(1024,)
  q,k,v = split_heads(x@W*), rope(q), rope(k), causal softmax(q k^T/8) v, @Wo+bo

Sharding: head-parallel across 8 cores. Core c owns heads {2c, 2c+1} for all
4 batches: it computes q/k/v projections against the 128-column weight slice,
attention for its heads, and a partial output projection against the matching
128-row slice of Wo. Host sums the 8 partial (8192,1024) outputs and adds bo.

On-core layout (all "T" tensors are feature-major: partitions=feature rows,
free=tokens):
  Q^T/K^T (128 x 2048/batch): rows = [h0 d-evens(32), h0 d-odds(32), h1 ...]
    (NeoX-style d-permutation, folded into the host-permuted weight columns;
     valid because q and k get the same permutation and qk^T is d-invariant)
  RoPE: Q <- Q*cos + (P2@Q)*sin2, where P2 swaps the even/odd halves per head
    (PE matmul) and sin2 carries the sign; DVE reads the projection PSUM
    directly (no staging copy).
  S^T tiles (tj x ti) = K^T.T @ Q^T per head (K=64 contraction, the PE
    double-pumps K<=64 fp16 so these stream at ~2 cols/cycle).
  A = exp(0.125*S^T) (ACT, one call covers both heads; straddle tiles
    band-masked with -1e30 triangle beforehand on DVE).
  O~^T (65 x ti) accumulated = [V|1].T @ A over tj chunks; row 64 = softmax
    denominators (ones column trick). Normalize: DVE reciprocal of the denom
    row, GPSIMD partition-broadcast, DVE multiply -> O^T rows (no DRAM
    round-trip).
  y partial (128t x 1024) = O^T-chunk.T @ Wo-slice, DMA'd psum->DRAM (fp32).

Scheduling: the attention inner loop is software-pipelined per 128-row K/V
chunk j (QK leads by 2 steps, exp by 1, AV trails); a cost-paced filler queue
interleaves the next batch's projection work and deferred output-projection
tiles into the attention steps so the PE never idles (which would also drop
its DVFS p-state).
"""

import numpy as np

B, T, C = 4, 2048, 1024
H, D = 16, 64
N_CORES = 8
BT = B * T
SCALE = 0.125  # D**-0.5
NEG = -1.0e30

TRACE = False            # set True (e.g. from test.py) to capture an NTFF trace
LAST_RESULT = None       # BassKernelResults of the most recent run

_BUILT = None            # cached nc


# --------------------------------------------------------------------------
# workaround: this walrus build rejects >1 semaphore wait per instruction
def _split_sem_waits(nc, max_waits=1):
    import concourse.mybir as mybir

    n = 0
    for f in nc.m.functions:
        for bb in f.blocks:
            insts = bb.instructions
            idx = 0
            while idx < len(insts):
                i = insts[idx]
                si = getattr(i, "sync_info", None)
                if si is not None and si.on_wait and len(si.on_wait) > max_waits:
                    waits = list(si.on_wait)
                    extra, keep = waits[:-max_waits], waits[-max_waits:]
                    si.on_wait = keep
                    pos = idx
                    for j in range(0, len(extra), max_waits):
                        n += 1
                        nd = mybir.InstNoOp(name=f"I-waitsplit-{n}", ins=[], outs=[])
                        nd.engine = i.engine
                        nd.sync_info = mybir.SyncInfo(
                            on_wait=extra[j : j + max_waits], on_update=[]
                        )
                        insts.insert(pos, nd)
                        pos += 1
                    idx = pos
                idx += 1


def _install_ntff_hook():
    """The image's antenv lacks axon_hooks; synthesize it so trace=True works."""
    import sys
    import types

    if "antenv.axon_hooks" in sys.modules:
        return
    import antenv

    state = {"hook": None}
    mod = types.ModuleType("antenv.axon_hooks")
    mod.get_axon_ntff_profile_hook = lambda: state["hook"]
    mod.set_axon_ntff_profile_hook = lambda h: state.__setitem__("hook", h)
    sys.modules["antenv.axon_hooks"] = mod
    antenv.axon_hooks = mod
    try:
        from trn_agent_boot.trn_boot import _ntff_profile_via_ctypes

        state["hook"] = _ntff_profile_via_ctypes("/opt/axon/libaxon_pjrt.so")
    except Exception:
        state["hook"] = None


# --------------------------------------------------------------------------
def _build():
    import concourse.bass as bass
    import concourse.mybir as mybir
    from concourse.tile import TileContext

    F = mybir.dt.float32
    MD = mybir.dt.float16  # matmul operand dtype
    MULT = mybir.AluOpType.mult
    ADD = mybir.AluOpType.add
    SUB = mybir.AluOpType.subtract
    EXP = mybir.ActivationFunctionType.Exp

    nc = bass.Bass()

    # xTd: host-prearranged [128, 16 nb-blocks, 8 k-chunks, 512 cols] so each
    # (b, nb) projection tile is one contiguous 8KB-per-partition DMA read.
    xTd = nc.dram_tensor("xTd", (128, 16 * 8 * 512), MD, kind="ExternalInput")
    # w*: host-prearranged [128, 8 k-chunks, 128 cols], contiguous per row.
    wq = nc.dram_tensor("wq", (128, 8 * 128), MD, kind="ExternalInput")
    wk = nc.dram_tensor("wk", (128, 8 * 128), MD, kind="ExternalInput")
    wv = nc.dram_tensor("wv", (128, 8 * 128), MD, kind="ExternalInput")
    wo = nc.dram_tensor("wo", (128, C), MD, kind="ExternalInput")
    cosd = nc.dram_tensor("cos", (128, T), MD, kind="ExternalInput")
    sind = nc.dram_tensor("sin2", (128, T), MD, kind="ExternalInput")
    p2d = nc.dram_tensor("p2", (128, 128), MD, kind="ExternalInput")
    bandd = nc.dram_tensor("band01", (128, 256), MD, kind="ExternalInput")
    y = nc.dram_tensor("y", (BT, C), MD, kind="ExternalOutput")
    scr = nc.dram_tensor("scr", (B * 8, 512), F, kind="Internal")

    with TileContext(nc) as tc:
        with (
            tc.tile_pool(name="const", bufs=1) as cst,
            tc.tile_pool(name="xt", bufs=3) as xtp,
            tc.tile_pool(name="qt", bufs=2) as qp,
            tc.tile_pool(name="kt", bufs=2) as kp,
            tc.tile_pool(name="vt", bufs=2) as vp,
            tc.tile_pool(name="ot", bufs=2) as op_,
            tc.tile_pool(name="vst", bufs=2) as vstp,
            tc.tile_pool(name="qs", bufs=2) as qsp,
            tc.tile_pool(name="at", bufs=4) as ap_,
            tc.tile_pool(name="rr", bufs=4) as rrp,
            tc.tile_pool(name="bc", bufs=4) as bcp,
            tc.tile_pool(name="ys", bufs=4) as ysp,
            tc.tile_pool(name="sps", bufs=2, space="PSUM") as sps,
            tc.tile_pool(name="stp", bufs=2, space="PSUM") as stp,
            tc.tile_pool(name="avp", bufs=2, space="PSUM") as avp,
        ):
            # ---- constants (emission order = DMA need order) ---------------
            wq_t = cst.tile([128, 8, 128], MD)
            nc.sync.dma_start(out=wq_t, in_=wq[:, :].rearrange(
                "p (a c) -> p a c", a=8))
            cos_t = cst.tile([128, T], MD)
            nc.sync.dma_start(out=cos_t, in_=cosd[:, :])
            sin_t = cst.tile([128, T], MD)
            nc.sync.dma_start(out=sin_t, in_=sind[:, :])
            wk_t = cst.tile([128, 8, 128], MD)
            nc.sync.dma_start(out=wk_t, in_=wk[:, :].rearrange(
                "p (a c) -> p a c", a=8))
            wv_t = cst.tile([128, 8, 128], MD)
            nc.sync.dma_start(out=wv_t, in_=wv[:, :].rearrange(
                "p (a c) -> p a c", a=8))
            p2_t = cst.tile([128, 128], MD)
            nc.sync.dma_start(out=p2_t, in_=p2d[:, :])
            band_t = cst.tile([128, 256], MD)  # 0/1 mask, [band | band]
            nc.sync.dma_start(out=band_t, in_=bandd[:, :])
            wo_t = cst.tile([128, C], MD)
            nc.sync.dma_start(out=wo_t, in_=wo[:, :])
            band2 = band_t[:, :].rearrange("p (a c) -> p a c", a=2)

            QKV = {}   # b -> (Qb, Kb, Vb)
            xts = {}   # (b, nb) -> xt tile

            def alloc_batch(b):
                Qb = qp.tile([128, T], MD, name="Qb")
                Kb = kp.tile([128, T], MD, name="Kb")
                # per (token-tile, head) an aligned 128-col slot:
                # [d 0..63 | ones | 63 unread] (XBAR transpose needs the
                # 256B-aligned output offsets)
                Vb = vp.tile([128, 16, 2, 128], MD, name="Vb")
                QKV[b] = (Qb, Kb, Vb)
                nc.gpsimd.memset(Vb[:, :, :, 64], 1.0)

            # ---- projection unit: 512 tokens of batch b --------------------
            def proj_items(b, nb):
                """Returns a list of (cost_ns, fn) items; fn emits instrs."""
                Qb, Kb, Vb = QKV[b]
                g0 = b * T + nb * 512
                cols = slice(nb * 512, (nb + 1) * 512)
                items = []

                def it_xt():
                    xt = xtp.tile([128, 8, 512], MD, name="xt")
                    xts[(b, nb)] = xt
                    g = (b * 4 + nb) * 4096
                    nc.sync.dma_start(
                        out=xt,
                        in_=xTd[:, g : g + 4096].rearrange(
                            "p (a c) -> p a c", a=8))
                items.append((50, it_xt))

                state = {}

                def mk_ps(W):
                    def fn():
                        xt = xts[(b, nb)]
                        ps = sps.tile([128, 512], F, tag="p", name="ps")
                        for k in range(8):
                            nc.tensor.matmul(
                                ps[:, :], lhsT=W[:, k, :], rhs=xt[:, k, :],
                                start=(k == 0), stop=(k == 7),
                            )
                        state["ps"] = ps
                    return fn

                def mk_rope_mul(dst):
                    def fn():
                        ps = state["ps"]
                        qs = qsp.tile([128, 512], MD, name="qs")
                        nc.vector.tensor_tensor(qs[:, :], ps[:, :],
                                                sin_t[:, cols], MULT)
                        nc.vector.tensor_tensor(dst[:, cols], ps[:, :],
                                                cos_t[:, cols], MULT)
                        state["qs"] = qs
                    return fn

                def mk_rope_rot(dst):
                    def fn():
                        qs = state.pop("qs")
                        state.pop("ps")
                        rot = sps.tile([128, 512], F, tag="p", name="rot")
                        nc.tensor.matmul(rot[:, :], lhsT=p2_t[:, :],
                                         rhs=qs[:, :], start=True, stop=True)
                        nc.vector.tensor_tensor(dst[:, cols], dst[:, cols],
                                                rot[:, :], SUB)
                    return fn

                items.append((1750, mk_ps(wq_t)))
                items.append((100, mk_rope_mul(Qb)))
                items.append((300, mk_rope_rot(Qb)))
                items.append((1750, mk_ps(wk_t)))
                items.append((100, mk_rope_mul(Kb)))
                items.append((300, mk_rope_rot(Kb)))
                items.append((1750, mk_ps(wv_t)))

                def it_vst():
                    # two partition-0 tiles (the XBAR transpose silently
                    # drops partition-offset sources)
                    ps = state.pop("ps")
                    vst0 = vstp.tile([64, 512], MD, tag="v0", name="vst0")
                    nc.scalar.copy(vst0[:, :], ps[0:64, :])
                    vst1 = vstp.tile([64, 512], MD, tag="v1", name="vst1")
                    nc.scalar.copy(vst1[:, :], ps[64:128, :])
                    state["vst"] = (vst0, vst1)
                items.append((150, it_vst))

                def mk_tr(tl):
                    def fn():
                        # XBAR DMA transpose: [feat 64, tok 128] fp16 ->
                        # token-major, landing per-head in Vb's aligned slots
                        vst0, vst1 = state["vst"]
                        tt = nb * 4 + tl
                        tcs = slice(tl * 128, (tl + 1) * 128)
                        nc.sync.dma_start_transpose(
                            Vb[:, tt, 0, 0:64], vst0[:, tcs])
                        nc.sync.dma_start_transpose(
                            Vb[:, tt, 1, 0:64], vst1[:, tcs])
                    return fn
                for tl in range(4):
                    items.append((60, mk_tr(tl)))
                return items

            # ---- output projection tile (128 tokens) -----------------------
            def mk_ytile(b, Ob, tt):
                def fn():
                    r0 = b * T + tt * 128
                    lhs = Ob[:, tt * 128 : (tt + 1) * 128]
                    ysb = ysp.tile([128, 1024], MD, name="ysb")
                    for nh in (0, 1):
                        yps = sps.tile([128, 512], F, tag="p", name="yps")
                        nc.tensor.matmul(
                            yps[:, :], lhsT=lhs,
                            rhs=wo_t[:, nh * 512 : (nh + 1) * 512],
                            start=True, stop=True,
                        )
                        dst = ysb[:, nh * 512 : (nh + 1) * 512]
                        nc.vector.tensor_copy(dst, yps[:, :])
                    nc.sync.dma_start(out=y[r0 : r0 + 128, :], in_=ysb[:, :])
                return fn

            # ---- filler queue ---------------------------------------------
            fillq = []   # list of (tag, cost, fn)

            def enq(tag, items):
                for cost, fn in items:
                    fillq.append((tag, cost, fn))

            def drain(pred):
                """Emit every queued item whose tag satisfies pred."""
                rest = []
                for tag, cost, fn in fillq:
                    if pred(tag):
                        fn()
                    else:
                        rest.append((tag, cost, fn))
                fillq[:] = rest

            def pop_budget(budget, cur_b):
                # y-tiles of batches >= 2 are reserved to feed the PE during
                # the fillerless last batch
                idx = 0
                while idx < len(fillq) and budget > 0:
                    tag, cost, fn = fillq[idx]
                    if tag[0] == "y" and tag[1] >= 2 and cur_b < 3:
                        idx += 1
                        continue
                    fillq.pop(idx)
                    fn()
                    budget -= cost
                return budget

            # ---- attention for batch b ------------------------------------
            def phase_d(b):
                Qb, Kb, Vb = QKV[b]
                Ob = op_.tile([128, T], MD, name="Ob")
                for i in range(4):
                    # block i must see its projections emitted already
                    drain(lambda t, b=b, i=i: t[0] == "p" and
                          (t[1], t[2]) <= (b, i))
                    nch = 4 * i + 4
                    avh = [avp.tile([128, 512], F, name="av") for _ in (0, 1)]
                    sts = {}
                    As = {}

                    def emit_qk(j, i=i, sts=sts):
                        delta = j * 128 - i * 512
                        nl = 512 - max(0, delta)
                        off = 512 - nl
                        st = stp.tile([128, 2, 512], F, name="st")
                        for h in (0, 1):
                            hs = slice(64 * h, 64 * h + 64)
                            nc.tensor.matmul(
                                st[:, h, 0:nl],
                                lhsT=Kb[hs, j * 128 : (j + 1) * 128],
                                rhs=Qb[hs, i * 512 + off : (i + 1) * 512],
                                start=True, stop=True,
                            )
                        sts[j] = (st, off, nl, delta >= 0)

                    def emit_exp(j, sts=sts, As=As):
                        st, off, nl, straddle = sts.pop(j)
                        A = ap_.tile([128, 2, 512], MD, name="A")
                        nc.scalar.activation(
                            A[:, :, 0:nl], st[:, :, 0:nl], EXP, scale=SCALE)
                        if straddle:
                            # zero the above-diagonal triangle (fp16 0/1 mask
                            # multiply on the otherwise-idle Pool engine)
                            nc.gpsimd.tensor_tensor(
                                A[:, :, 0:128], A[:, :, 0:128], band2, MULT)
                        As[j] = (A, off, nl)

                    def emit_av(j, nch=nch, As=As, avh=avh):
                        A, off, nl = As.pop(j)
                        for h in (0, 1):
                            nc.tensor.matmul(
                                avh[h][0:65, off:512],
                                lhsT=Vb[:, j, h, 0:65],
                                rhs=A[:, h, 0:nl],
                                start=(j == 0), stop=(j == nch - 1),
                                skip_group_check=True,
                            )

                    for s in range(nch + 2):
                        if s < nch:
                            emit_qk(s)
                        if 1 <= s <= nch:
                            emit_exp(s - 1)
                        if s >= 2:
                            emit_av(s - 2)
                        pop_budget(700, b)

                    # normalize: O = O~ / denom (denom in row 64 of av).
                    # ACT evacuates the denom row, SBUF->SBUF DMA repartitions
                    # it to [128,4] so the DVE reciprocal runs all lanes, then
                    # a DRAM bounce broadcasts it across 64 partitions
                    # (stride-0 partition reads are only legal from DRAM).
                    for h in (0, 1):
                        row = b * 8 + i * 2 + h
                        rden = rrp.tile([1, 512], F, name="rden")
                        nc.scalar.copy(rden[:, :], avh[h][64:65, 0:512])
                        srt = rrp.tile([128, 4], F, name="srt")
                        nc.sync.dma_start(out=srt[:, :], in_=rden[:, :])
                        rt = rrp.tile([128, 4], F, name="rt")
                        nc.vector.reciprocal(rt[:, :], srt[:, :])
                        nc.sync.dma_start(
                            out=scr[row : row + 1, :].rearrange(
                                "r (p c) -> (r p) c", c=4),
                            in_=rt[:, :])
                        bct = bcp.tile([64, 512], F, name="bct")
                        src = scr[row : row + 1, :]
                        bap = bass.AP(
                            tensor=src.tensor, offset=src.offset,
                            ap=[[0, 64]] + [list(p) for p in src.ap[1:]],
                        )
                        nc.sync.dma_start(out=bct[:, :], in_=bap)
                        nc.vector.tensor_tensor(
                            Ob[64 * h : 64 * h + 64, i * 512 : (i + 1) * 512],
                            avh[h][0:64, 0:512], bct[:, :], MULT)

                    # defer the 4 output tiles of this block via the queue
                    for tt in range(4 * i, 4 * i + 4):
                        fillq.append((("y", b, i), 700, mk_ytile(b, Ob, tt)))

            # ---- top-level schedule ---------------------------------------
            alloc_batch(0)
            for cost, fn in proj_items(0, 0):
                fn()                      # first unit inline: critical path
            for nb in range(1, 4):
                enq(("p", 0, nb), proj_items(0, nb))
            for b in range(B):
                if b + 1 < B:
                    alloc_batch(b + 1)
                    for nb in range(4):
                        enq(("p", b + 1, nb), proj_items(b + 1, nb))
                phase_d(b)
            drain(lambda t: True)

    _split_sem_waits(nc)
    return nc


# --------------------------------------------------------------------------
def _host_inputs(x, Wq, Wk, Wv, Wo):
    """Per-core input dicts (all shared arrays built once)."""
    BF = np.float16

    # x^T arranged [p, nb-block, k-chunk, col] so each projection tile is one
    # contiguous 8KB-per-partition DMA read
    xT = np.asarray(x, dtype=np.float32).reshape(BT, C).T  # (C, BT)
    xTd = np.ascontiguousarray(
        xT.reshape(8, 128, 16, 512).transpose(1, 2, 0, 3).reshape(128, -1)
    ).astype(BF)

    # NeoX d-permutation within each head: evens then odds
    dperm = np.concatenate([np.arange(0, D, 2), np.arange(1, D, 2)])

    inv_freq = (1.0 / (10000.0 ** (np.arange(0, D, 2) / D))).astype(np.float64)
    pos = np.arange(T, dtype=np.float64)
    ang = pos[None, :] * inv_freq[:, None]  # (32, T)
    cos32 = np.cos(ang).astype(np.float32)
    sin32 = np.sin(ang).astype(np.float32)
    cos_t = np.tile(np.vstack([cos32, cos32]), (2, 1))  # (128, T)
    sin_t = np.tile(np.vstack([-sin32, sin32]), (2, 1))  # (128, T), sign folded

    p2 = np.zeros((128, 128), dtype=np.float32)
    for hb in (0, 64):
        for i2 in range(32):
            p2[hb + i2, hb + 32 + i2] = 1.0
            p2[hb + 32 + i2, hb + i2] = 1.0

    # 0/1 causal mask (1 where q-offset >= k-offset), doubled for head pairs
    band = np.where(
        np.arange(128)[None, :] >= np.arange(128)[:, None], 1.0, 0.0
    ).astype(np.float32)
    band01 = np.concatenate([band, band], axis=1)  # (128, 256)

    def wlayout(w):  # (C, 128) -> (128, 8*128), row-contiguous per k-chunk
        return np.ascontiguousarray(
            w.reshape(8, 128, 128).transpose(1, 0, 2).reshape(128, -1)
        ).astype(BF)

    Wq = np.asarray(Wq, dtype=np.float32)
    Wk = np.asarray(Wk, dtype=np.float32)
    Wv = np.asarray(Wv, dtype=np.float32)
    Wo = np.asarray(Wo, dtype=np.float32)

    in_maps = []
    for c in range(N_CORES):
        sl = slice(128 * c, 128 * (c + 1))
        wq_c = Wq[:, sl].reshape(C, 2, D)[:, :, dperm].reshape(C, 128)
        wk_c = Wk[:, sl].reshape(C, 2, D)[:, :, dperm].reshape(C, 128)
        in_maps.append({
            "xTd": xTd,
            "wq": wlayout(wq_c),
            "wk": wlayout(wk_c),
            "wv": wlayout(Wv[:, sl]),
            "wo": np.ascontiguousarray(Wo[sl, :]).astype(BF),
            "cos": cos_t.astype(BF),
            "sin2": sin_t.astype(BF),
            "p2": p2.astype(BF),
            "band01": band01.astype(BF),
        })
    return in_maps


def kernel(x, Wq, Wk, Wv, Wo, bo):
    global _BUILT, LAST_RESULT
    from concourse.bass_utils import run_bass_kernel_spmd

    if TRACE:
        _install_ntff_hook()

    if _BUILT is None:
        _BUILT = _build()
    nc = _BUILT

    in_maps = _host_inputs(x, Wq, Wk, Wv, Wo)

    last_err = None
    for attempt in range(3):
        try:
            res = run_bass_kernel_spmd(
                nc, in_maps, core_ids=list(range(N_CORES)), trace=TRACE
            )
            break
        except Exception as e:  # transient NRT device errors: retry
            last_err = e
            import time as _time

            _time.sleep(2.0)
    else:
        raise last_err
    LAST_RESULT = res

    acc = res.results[0]["y"].astype(np.float64)
    for c in range(1, N_CORES):
        acc = acc + res.results[c]["y"]
    out = acc.astype(np.float32) + np.asarray(bo, dtype=np.float32)[None, :]
    return out.reshape(B, T, C)


# revision 28
# speedup vs baseline: 1.0932x; 1.0932x over previous
"""Causal multi-head self-attention (RoPE) Trainium2 Bass kernel.

Problem: x:(4,2048,1024), Wq/Wk/Wv:(1024,1024), Wo:(1024,1024), bo:(1024,)
  q,k,v = split_heads(x@W*), rope(q), rope(k), causal softmax(q k^T/8) v, @Wo+bo

Sharding: head-parallel across 8 cores. Core c owns heads {2c, 2c+1} for all
4 batches: it computes q/k/v projections against the 128-column weight slice,
attention for its heads, and a partial output projection against the matching
128-row slice of Wo. Host sums the 8 partial (8192,1024) outputs and adds bo.

On-core layout (all "T" tensors are feature-major: partitions=feature rows,
free=tokens):
  Q^T/K^T (128 x 2048/batch): rows = [h0 d-evens(32), h0 d-odds(32), h1 ...]
    (NeoX-style d-permutation, folded into the host-permuted weight columns;
     valid because q and k get the same permutation and qk^T is d-invariant)
  RoPE: Q <- Q*cos + (P2@Q)*sin2, where P2 swaps the even/odd halves per head
    (PE matmul) and sin2 carries the sign; DVE reads the projection PSUM
    directly (no staging copy).
  S^T tiles (tj x ti) = K^T.T @ Q^T per head (K=64 contraction, the PE
    double-pumps K<=64 fp16 so these stream at ~2 cols/cycle).
  A = exp(0.125*S^T) (ACT, one call covers both heads; straddle tiles
    band-masked with -1e30 triangle beforehand on DVE).
  O~^T (65 x ti) accumulated = [V|1].T @ A over tj chunks; row 64 = softmax
    denominators (ones column trick). Normalize: DVE reciprocal of the denom
    row, GPSIMD partition-broadcast, DVE multiply -> O^T rows (no DRAM
    round-trip).
  y partial (128t x 1024) = O^T-chunk.T @ Wo-slice, DMA'd psum->DRAM (fp32).

Scheduling: the attention inner loop is software-pipelined per 128-row K/V
chunk j (QK leads by 2 steps, exp by 1, AV trails); a cost-paced filler queue
interleaves the next batch's projection work and deferred output-projection
tiles into the attention steps so the PE never idles (which would also drop
its DVFS p-state).
"""

import numpy as np

B, T, C = 4, 2048, 1024
H, D = 16, 64
N_CORES = 8
BT = B * T
SCALE = 0.125  # D**-0.5
NEG = -1.0e30

TRACE = False            # set True (e.g. from test.py) to capture an NTFF trace
LAST_RESULT = None       # BassKernelResults of the most recent run

_BUILT = None            # cached nc


# --------------------------------------------------------------------------
# workaround: this walrus build rejects >1 semaphore wait per instruction
def _split_sem_waits(nc, max_waits=1):
    import concourse.mybir as mybir

    n = 0
    for f in nc.m.functions:
        for bb in f.blocks:
            insts = bb.instructions
            idx = 0
            while idx < len(insts):
                i = insts[idx]
                si = getattr(i, "sync_info", None)
                if si is not None and si.on_wait and len(si.on_wait) > max_waits:
                    waits = list(si.on_wait)
                    extra, keep = waits[:-max_waits], waits[-max_waits:]
                    si.on_wait = keep
                    pos = idx
                    for j in range(0, len(extra), max_waits):
                        n += 1
                        nd = mybir.InstNoOp(name=f"I-waitsplit-{n}", ins=[], outs=[])
                        nd.engine = i.engine
                        nd.sync_info = mybir.SyncInfo(
                            on_wait=extra[j : j + max_waits], on_update=[]
                        )
                        insts.insert(pos, nd)
                        pos += 1
                    idx = pos
                idx += 1


def _install_ntff_hook():
    """The image's antenv lacks axon_hooks; synthesize it so trace=True works."""
    import sys
    import types

    if "antenv.axon_hooks" in sys.modules:
        return
    import antenv

    state = {"hook": None}
    mod = types.ModuleType("antenv.axon_hooks")
    mod.get_axon_ntff_profile_hook = lambda: state["hook"]
    mod.set_axon_ntff_profile_hook = lambda h: state.__setitem__("hook", h)
    sys.modules["antenv.axon_hooks"] = mod
    antenv.axon_hooks = mod
    try:
        from trn_agent_boot.trn_boot import _ntff_profile_via_ctypes

        state["hook"] = _ntff_profile_via_ctypes("/opt/axon/libaxon_pjrt.so")
    except Exception:
        state["hook"] = None


# --------------------------------------------------------------------------
def _build():
    import concourse.bass as bass
    import concourse.mybir as mybir
    from concourse.tile import TileContext

    F = mybir.dt.float32
    MD = mybir.dt.float16  # matmul operand dtype
    MULT = mybir.AluOpType.mult
    ADD = mybir.AluOpType.add
    SUB = mybir.AluOpType.subtract
    EXP = mybir.ActivationFunctionType.Exp

    nc = bass.Bass()

    # xTd: host-prearranged [128, 16 nb-blocks, 8 k-chunks, 512 cols] so each
    # (b, nb) projection tile is one contiguous 8KB-per-partition DMA read.
    xTd = nc.dram_tensor("xTd", (128, 16 * 8 * 512), MD, kind="ExternalInput")
    # w*: host-prearranged [128, 8 k-chunks, 128 cols], contiguous per row.
    wq = nc.dram_tensor("wq", (128, 8 * 128), MD, kind="ExternalInput")
    wk = nc.dram_tensor("wk", (128, 8 * 128), MD, kind="ExternalInput")
    wv = nc.dram_tensor("wv", (128, 8 * 128), MD, kind="ExternalInput")
    wo = nc.dram_tensor("wo", (128, C), MD, kind="ExternalInput")
    cosd = nc.dram_tensor("cos", (128, T), MD, kind="ExternalInput")
    sind = nc.dram_tensor("sin2", (128, T), MD, kind="ExternalInput")
    p2d = nc.dram_tensor("p2", (128, 128), MD, kind="ExternalInput")
    bandd = nc.dram_tensor("band01", (128, 256), MD, kind="ExternalInput")
    y = nc.dram_tensor("y", (BT, C), MD, kind="ExternalOutput")
    scr = nc.dram_tensor("scr", (B * 8, 512), F, kind="Internal")

    with TileContext(nc) as tc:
        with (
            tc.tile_pool(name="const", bufs=1) as cst,
            tc.tile_pool(name="xt", bufs=3) as xtp,
            tc.tile_pool(name="qt", bufs=2) as qp,
            tc.tile_pool(name="kt", bufs=2) as kp,
            tc.tile_pool(name="vt", bufs=2) as vp,
            tc.tile_pool(name="ot", bufs=2) as op_,
            tc.tile_pool(name="vst", bufs=2) as vstp,
            tc.tile_pool(name="qs", bufs=2) as qsp,
            tc.tile_pool(name="at", bufs=4) as ap_,
            tc.tile_pool(name="rr", bufs=4) as rrp,
            tc.tile_pool(name="bc", bufs=4) as bcp,
            tc.tile_pool(name="ys", bufs=4) as ysp,
            tc.tile_pool(name="avs", bufs=2) as avsp,
            tc.tile_pool(name="sps", bufs=2, space="PSUM") as sps,
            tc.tile_pool(name="stp", bufs=2, space="PSUM") as stp,
            tc.tile_pool(name="avp", bufs=2, space="PSUM") as avp,
        ):
            # ---- constants (emission order = DMA need order) ---------------
            wq_t = cst.tile([128, 8, 128], MD)
            nc.sync.dma_start(out=wq_t, in_=wq[:, :].rearrange(
                "p (a c) -> p a c", a=8))
            cos_t = cst.tile([128, T], MD)
            nc.sync.dma_start(out=cos_t, in_=cosd[:, :])
            sin_t = cst.tile([128, T], MD)
            nc.sync.dma_start(out=sin_t, in_=sind[:, :])
            wk_t = cst.tile([128, 8, 128], MD)
            nc.sync.dma_start(out=wk_t, in_=wk[:, :].rearrange(
                "p (a c) -> p a c", a=8))
            wv_t = cst.tile([128, 8, 128], MD)
            nc.sync.dma_start(out=wv_t, in_=wv[:, :].rearrange(
                "p (a c) -> p a c", a=8))
            p2_t = cst.tile([128, 128], MD)
            nc.sync.dma_start(out=p2_t, in_=p2d[:, :])
            band_t = cst.tile([128, 256], MD)  # 0/1 mask, [band | band]
            nc.sync.dma_start(out=band_t, in_=bandd[:, :])
            wo_t = cst.tile([128, C], MD)
            nc.sync.dma_start(out=wo_t, in_=wo[:, :])
            band2 = band_t[:, :].rearrange("p (a c) -> p a c", a=2)

            QKV = {}   # b -> (Qb, Kb, Vb)
            xts = {}   # (b, nb) -> xt tile

            def alloc_batch(b):
                Qb = qp.tile([128, T], MD, name="Qb")
                Kb = kp.tile([128, T], MD, name="Kb")
                # per (token-tile, head) an aligned 128-col slot:
                # [d 0..63 | ones | 63 unread] (XBAR transpose needs the
                # 256B-aligned output offsets)
                Vb = vp.tile([128, 16, 2, 128], MD, name="Vb")
                QKV[b] = (Qb, Kb, Vb)
                nc.gpsimd.memset(Vb[:, :, :, 64], 1.0)

            # ---- projection unit: 512 tokens of batch b --------------------
            def xt_item(b, nb):
                def it_xt():
                    xt = xtp.tile([128, 8, 512], MD, name="xt")
                    xts[(b, nb)] = xt
                    g = (b * 4 + nb) * 4096
                    # 4 DMAs on separate queues: ~4x faster tile arrival
                    for q in range(4):
                        nc.sync.dma_start(
                            out=xt[:, 2 * q : 2 * q + 2, :],
                            in_=xTd[:, g + 1024 * q : g + 1024 * (q + 1)]
                            .rearrange("p (a c) -> p a c", a=2))
                return (50, it_xt)

            def proj_items(b, nb):
                """Returns a list of (cost_ns, fn) items; fn emits instrs.
                The unit's xt DMA is NOT included (prefetched separately)."""
                Qb, Kb, Vb = QKV[b]
                cols = slice(nb * 512, (nb + 1) * 512)
                items = []

                state = {}

                def mk_ps(W):
                    def fn():
                        xt = xts[(b, nb)]
                        ps = sps.tile([128, 512], F, tag="p", name="ps")
                        for k in range(8):
                            nc.tensor.matmul(
                                ps[:, :], lhsT=W[:, k, :], rhs=xt[:, k, :],
                                start=(k == 0), stop=(k == 7),
                            )
                        state["ps"] = ps
                    return fn

                def mk_rope_mul(dst):
                    def fn():
                        ps = state["ps"]
                        qs = qsp.tile([128, 512], MD, name="qs")
                        nc.vector.tensor_tensor(qs[:, :], ps[:, :],
                                                sin_t[:, cols], MULT)
                        nc.vector.tensor_tensor(dst[:, cols], ps[:, :],
                                                cos_t[:, cols], MULT)
                        state["qs"] = qs
                    return fn

                def mk_rope_rot(dst):
                    def fn():
                        qs = state.pop("qs")
                        state.pop("ps")
                        rot = sps.tile([128, 512], F, tag="p", name="rot")
                        nc.tensor.matmul(rot[:, :], lhsT=p2_t[:, :],
                                         rhs=qs[:, :], start=True, stop=True)
                        nc.vector.tensor_tensor(dst[:, cols], dst[:, cols],
                                                rot[:, :], SUB)
                    return fn

                items.append((1750, mk_ps(wq_t)))
                items.append((100, mk_rope_mul(Qb)))
                items.append((300, mk_rope_rot(Qb)))
                items.append((1750, mk_ps(wk_t)))
                items.append((100, mk_rope_mul(Kb)))
                items.append((300, mk_rope_rot(Kb)))
                items.append((1750, mk_ps(wv_t)))

                def it_vst():
                    # two partition-0 tiles (the XBAR transpose silently
                    # drops partition-offset sources)
                    ps = state.pop("ps")
                    vst0 = vstp.tile([64, 512], MD, tag="v0", name="vst0")
                    nc.scalar.copy(vst0[:, :], ps[0:64, :])
                    vst1 = vstp.tile([64, 512], MD, tag="v1", name="vst1")
                    nc.scalar.copy(vst1[:, :], ps[64:128, :])
                    state["vst"] = (vst0, vst1)
                items.append((150, it_vst))

                def mk_tr(tl):
                    def fn():
                        # XBAR DMA transpose: [feat 64, tok 128] fp16 ->
                        # token-major, landing per-head in Vb's aligned slots
                        vst0, vst1 = state["vst"]
                        tt = nb * 4 + tl
                        tcs = slice(tl * 128, (tl + 1) * 128)
                        nc.sync.dma_start_transpose(
                            Vb[:, tt, 0, 0:64], vst0[:, tcs])
                        nc.sync.dma_start_transpose(
                            Vb[:, tt, 1, 0:64], vst1[:, tcs])
                    return fn
                for tl in range(4):
                    items.append((60, mk_tr(tl)))
                return items



            # ---- output projection tile (128 tokens) -----------------------
            def mk_ytile(b, Ob, tt):
                def fn():
                    r0 = b * T + tt * 128
                    lhs = Ob[:, tt * 128 : (tt + 1) * 128]
                    ysb = ysp.tile([128, 1024], MD, name="ysb")
                    for nh in (0, 1):
                        yps = sps.tile([128, 512], F, tag="p", name="yps")
                        nc.tensor.matmul(
                            yps[:, :], lhsT=lhs,
                            rhs=wo_t[:, nh * 512 : (nh + 1) * 512],
                            start=True, stop=True,
                        )
                        dst = ysb[:, nh * 512 : (nh + 1) * 512]
                        nc.vector.tensor_copy(dst, yps[:, :])
                    nc.sync.dma_start(out=y[r0 : r0 + 128, :], in_=ysb[:, :])
                return fn

            # ---- filler queue ---------------------------------------------
            fillq = []   # list of (tag, cost, fn)

            def enq(tag, items):
                for cost, fn in items:
                    fillq.append((tag, cost, fn))

            def drain(pred):
                """Emit every queued item whose tag satisfies pred."""
                rest = []
                for tag, cost, fn in fillq:
                    if pred(tag):
                        fn()
                    else:
                        rest.append((tag, cost, fn))
                fillq[:] = rest

            def pop_budget(budget, cur_b):
                # y-tiles of batches >= 2 are reserved to feed the PE during
                # the fillerless last batch
                idx = 0
                while idx < len(fillq) and budget > 0:
                    tag, cost, fn = fillq[idx]
                    if tag[0] == "y" and tag[1] >= 2 and cur_b < 3:
                        idx += 1
                        continue
                    fillq.pop(idx)
                    fn()
                    budget -= cost
                return budget

            # ---- attention for batch b ------------------------------------
            def norm_group(b, g, avs8, Ob):
                """Batched normalize for i-blocks {2g, 2g+1}: one repartition
                DMA + all-lane reciprocal + one writeback, then a stride-0
                DRAM broadcast read and the O multiply per (i, h)."""
                r0 = b * 8 + g * 4
                srt = rrp.tile([128, 16], F, name="srt")
                nc.sync.dma_start(
                    out=srt[:, :], in_=avs8[64:65, 4 * g : 4 * g + 4, :])
                rt = rrp.tile([128, 16], F, name="rt")
                nc.vector.reciprocal(rt[:, :], srt[:, :])
                nc.sync.dma_start(
                    out=scr[r0 : r0 + 4, :].rearrange("a c -> (a c)"),
                    in_=rt[:, :])
                for ih in range(4):
                    i, h = 2 * g + ih // 2, ih % 2
                    bct = bcp.tile([64, 512], F, name="bct")
                    src = scr[r0 + ih : r0 + ih + 1, :]
                    bap = bass.AP(
                        tensor=src.tensor, offset=src.offset,
                        ap=[[0, 64]] + [list(p) for p in src.ap[1:]],
                    )
                    nc.sync.dma_start(out=bct[:, :], in_=bap)
                    nc.vector.tensor_tensor(
                        Ob[64 * h : 64 * h + 64, i * 512 : (i + 1) * 512],
                        avs8[0:64, 4 * g + ih, :], bct[:, :], MULT)

            def phase_d(b):
                Qb, Kb, Vb = QKV[b]
                Ob = op_.tile([128, T], MD, name="Ob")
                avs8 = avsp.tile([65, 8, 512], F, name="avs8")
                for i in range(4):
                    # block i must see its projections emitted already
                    drain(lambda t, b=b, i=i: t[0] == "p" and
                          (t[1], t[2]) <= (b, i))
                    nch = 4 * i + 4
                    avh = [avp.tile([128, 512], F, name="av") for _ in (0, 1)]
                    sts = {}
                    As = {}

                    def emit_qk(j, i=i, sts=sts):
                        delta = j * 128 - i * 512
                        nl = 512 - max(0, delta)
                        off = 512 - nl
                        st = stp.tile([128, 2, 512], F, name="st")
                        for h in (0, 1):
                            hs = slice(64 * h, 64 * h + 64)
                            nc.tensor.matmul(
                                st[:, h, 0:nl],
                                lhsT=Kb[hs, j * 128 : (j + 1) * 128],
                                rhs=Qb[hs, i * 512 + off : (i + 1) * 512],
                                start=True, stop=True,
                            )
                        sts[j] = (st, off, nl, delta >= 0)

                    def emit_exp(j, sts=sts, As=As):
                        st, off, nl, straddle = sts.pop(j)
                        A = ap_.tile([128, 2, 512], MD, name="A")
                        nc.scalar.activation(
                            A[:, :, 0:nl], st[:, :, 0:nl], EXP, scale=SCALE)
                        if straddle:
                            # zero the above-diagonal triangle (fp16 0/1 mask
                            # multiply on the otherwise-idle Pool engine)
                            nc.gpsimd.tensor_tensor(
                                A[:, :, 0:128], A[:, :, 0:128], band2, MULT)
                        As[j] = (A, off, nl)

                    def emit_av(j, nch=nch, As=As, avh=avh):
                        A, off, nl = As.pop(j)
                        for h in (0, 1):
                            nc.tensor.matmul(
                                avh[h][0:65, off:512],
                                lhsT=Vb[:, j, h, 0:65],
                                rhs=A[:, h, 0:nl],
                                start=(j == 0), stop=(j == nch - 1),
                                skip_group_check=True,
                            )

                    for s in range(nch + 2):
                        if s < nch:
                            emit_qk(s)
                        if 1 <= s <= nch:
                            emit_exp(s - 1)
                        if s >= 2:
                            emit_av(s - 2)
                        pop_budget(700, b)

                    # evacuate O~ + denom row to the SBUF staging tile (frees
                    # the PSUM accumulators immediately; ACT/DVE split)
                    nc.scalar.copy(avs8[:, 2 * i, :], avh[0][0:65, 0:512])
                    nc.vector.tensor_copy(avs8[:, 2 * i + 1, :],
                                          avh[1][0:65, 0:512])

                    if i % 2 == 1:
                        norm_group(b, i // 2, avs8, Ob)
                        # defer the output tiles of both blocks via the queue
                        for tt in range(8 * (i // 2), 8 * (i // 2) + 8):
                            fillq.append(
                                (("y", b, i), 700, mk_ytile(b, Ob, tt)))

            # ---- top-level schedule ---------------------------------------
            def enq_batch(b):
                for nb in range(4):
                    unit = proj_items(b, nb)
                    if nb == 0:
                        unit.insert(0, xt_item(b, 0))
                    if nb + 1 < 4:
                        unit.insert(3, xt_item(b, nb + 1))
                    enq(("p", b, nb), unit)

            alloc_batch(0)
            xt_item(0, 0)[1]()
            xt_item(0, 1)[1]()
            for cost, fn in proj_items(0, 0):
                fn()                      # first unit inline: critical path
            for nb in range(1, 4):
                unit = proj_items(0, nb)
                if nb + 1 < 4:
                    unit.insert(2, xt_item(0, nb + 1))
                enq(("p", 0, nb), unit)
            for b in range(B):
                if b + 1 < B:
                    alloc_batch(b + 1)
                    enq_batch(b + 1)
                phase_d(b)
            drain(lambda t: True)

    _split_sem_waits(nc)
    return nc


# --------------------------------------------------------------------------
def _host_inputs(x, Wq, Wk, Wv, Wo):
    """Per-core input dicts (all shared arrays built once)."""
    BF = np.float16

    # x^T arranged [p, nb-block, k-chunk, col] so each projection tile is one
    # contiguous 8KB-per-partition DMA read
    xT = np.asarray(x, dtype=np.float32).reshape(BT, C).T  # (C, BT)
    xTd = np.ascontiguousarray(
        xT.reshape(8, 128, 16, 512).transpose(1, 2, 0, 3).reshape(128, -1)
    ).astype(BF)

    # NeoX d-permutation within each head: evens then odds
    dperm = np.concatenate([np.arange(0, D, 2), np.arange(1, D, 2)])

    inv_freq = (1.0 / (10000.0 ** (np.arange(0, D, 2) / D))).astype(np.float64)
    pos = np.arange(T, dtype=np.float64)
    ang = pos[None, :] * inv_freq[:, None]  # (32, T)
    cos32 = np.cos(ang).astype(np.float32)
    sin32 = np.sin(ang).astype(np.float32)
    cos_t = np.tile(np.vstack([cos32, cos32]), (2, 1))  # (128, T)
    sin_t = np.tile(np.vstack([-sin32, sin32]), (2, 1))  # (128, T), sign folded

    p2 = np.zeros((128, 128), dtype=np.float32)
    for hb in (0, 64):
        for i2 in range(32):
            p2[hb + i2, hb + 32 + i2] = 1.0
            p2[hb + 32 + i2, hb + i2] = 1.0

    # 0/1 causal mask (1 where q-offset >= k-offset), doubled for head pairs
    band = np.where(
        np.arange(128)[None, :] >= np.arange(128)[:, None], 1.0, 0.0
    ).astype(np.float32)
    band01 = np.concatenate([band, band], axis=1)  # (128, 256)

    def wlayout(w):  # (C, 128) -> (128, 8*128), row-contiguous per k-chunk
        return np.ascontiguousarray(
            w.reshape(8, 128, 128).transpose(1, 0, 2).reshape(128, -1)
        ).astype(BF)

    Wq = np.asarray(Wq, dtype=np.float32)
    Wk = np.asarray(Wk, dtype=np.float32)
    Wv = np.asarray(Wv, dtype=np.float32)
    Wo = np.asarray(Wo, dtype=np.float32)

    in_maps = []
    for c in range(N_CORES):
        sl = slice(128 * c, 128 * (c + 1))
        wq_c = Wq[:, sl].reshape(C, 2, D)[:, :, dperm].reshape(C, 128)
        wk_c = Wk[:, sl].reshape(C, 2, D)[:, :, dperm].reshape(C, 128)
        in_maps.append({
            "xTd": xTd,
            "wq": wlayout(wq_c),
            "wk": wlayout(wk_c),
            "wv": wlayout(Wv[:, sl]),
            "wo": np.ascontiguousarray(Wo[sl, :]).astype(BF),
            "cos": cos_t.astype(BF),
            "sin2": sin_t.astype(BF),
            "p2": p2.astype(BF),
            "band01": band01.astype(BF),
        })
    return in_maps


def kernel(x, Wq, Wk, Wv, Wo, bo):
    global _BUILT, LAST_RESULT
    from concourse.bass_utils import run_bass_kernel_spmd

    if TRACE:
        _install_ntff_hook()

    if _BUILT is None:
        _BUILT = _build()
    nc = _BUILT

    in_maps = _host_inputs(x, Wq, Wk, Wv, Wo)

    last_err = None
    for attempt in range(3):
        try:
            res = run_bass_kernel_spmd(
                nc, in_maps, core_ids=list(range(N_CORES)), trace=TRACE
            )
            break
        except Exception as e:  # transient NRT device errors: retry
            last_err = e
            import time as _time

            _time.sleep(2.0)
    else:
        raise last_err
    LAST_RESULT = res

    acc = res.results[0]["y"].astype(np.float64)
    for c in range(1, N_CORES):
        acc = acc + res.results[c]["y"]
    out = acc.astype(np.float32) + np.asarray(bo, dtype=np.float32)[None, :]
    return out.reshape(B, T, C)


# revision 30
# speedup vs baseline: 1.1678x; 1.0682x over previous
"""Causal multi-head self-attention (RoPE) Trainium2 Bass kernel.

Problem: x:(4,2048,1024), Wq/Wk/Wv:(1024,1024), Wo:(1024,1024), bo:(1024,)
  q,k,v = split_heads(x@W*), rope(q), rope(k), causal softmax(q k^T/8) v, @Wo+bo

Sharding: head-parallel across 8 cores. Core c owns heads {2c, 2c+1} for all
4 batches: it computes q/k/v projections against the 128-column weight slice,
attention for its heads, and a partial output projection against the matching
128-row slice of Wo. Host sums the 8 partial (8192,1024) outputs and adds bo.

On-core layout (all "T" tensors are feature-major: partitions=feature rows,
free=tokens):
  Q^T/K^T (128 x 2048/batch): rows = [h0 d-evens(32), h0 d-odds(32), h1 ...]
    (NeoX-style d-permutation, folded into the host-permuted weight columns;
     valid because q and k get the same permutation and qk^T is d-invariant)
  RoPE: Q <- Q*cos + (P2@Q)*sin2, where P2 swaps the even/odd halves per head
    (PE matmul) and sin2 carries the sign; DVE reads the projection PSUM
    directly (no staging copy).
  S^T tiles (tj x ti) = K^T.T @ Q^T per head (K=64 contraction, the PE
    double-pumps K<=64 fp16 so these stream at ~2 cols/cycle).
  A = exp(0.125*S^T) (ACT, one call covers both heads; straddle tiles
    band-masked with -1e30 triangle beforehand on DVE).
  O~^T (65 x ti) accumulated = [V|1].T @ A over tj chunks; row 64 = softmax
    denominators (ones column trick). Normalize: DVE reciprocal of the denom
    row, GPSIMD partition-broadcast, DVE multiply -> O^T rows (no DRAM
    round-trip).
  y partial (128t x 1024) = O^T-chunk.T @ Wo-slice, DMA'd psum->DRAM (fp32).

Scheduling: the attention inner loop is software-pipelined per 128-row K/V
chunk j (QK leads by 2 steps, exp by 1, AV trails); a cost-paced filler queue
interleaves the next batch's projection work and deferred output-projection
tiles into the attention steps so the PE never idles (which would also drop
its DVFS p-state).
"""

import numpy as np

B, T, C = 4, 2048, 1024
H, D = 16, 64
N_CORES = 8
BT = B * T
SCALE = 0.125  # D**-0.5
NEG = -1.0e30

TRACE = False            # set True (e.g. from test.py) to capture an NTFF trace
LAST_RESULT = None       # BassKernelResults of the most recent run

_BUILT = None            # cached nc


# --------------------------------------------------------------------------
# workaround: this walrus build rejects >1 semaphore wait per instruction
def _split_sem_waits(nc, max_waits=1):
    import concourse.mybir as mybir

    n = 0
    for f in nc.m.functions:
        for bb in f.blocks:
            insts = bb.instructions
            idx = 0
            while idx < len(insts):
                i = insts[idx]
                si = getattr(i, "sync_info", None)
                if si is not None and si.on_wait and len(si.on_wait) > max_waits:
                    waits = list(si.on_wait)
                    extra, keep = waits[:-max_waits], waits[-max_waits:]
                    si.on_wait = keep
                    pos = idx
                    for j in range(0, len(extra), max_waits):
                        n += 1
                        nd = mybir.InstNoOp(name=f"I-waitsplit-{n}", ins=[], outs=[])
                        nd.engine = i.engine
                        nd.sync_info = mybir.SyncInfo(
                            on_wait=extra[j : j + max_waits], on_update=[]
                        )
                        insts.insert(pos, nd)
                        pos += 1
                    idx = pos
                idx += 1


def _install_ntff_hook():
    """The image's antenv lacks axon_hooks; synthesize it so trace=True works."""
    import sys
    import types

    if "antenv.axon_hooks" in sys.modules:
        return
    import antenv

    state = {"hook": None}
    mod = types.ModuleType("antenv.axon_hooks")
    mod.get_axon_ntff_profile_hook = lambda: state["hook"]
    mod.set_axon_ntff_profile_hook = lambda h: state.__setitem__("hook", h)
    sys.modules["antenv.axon_hooks"] = mod
    antenv.axon_hooks = mod
    try:
        from trn_agent_boot.trn_boot import _ntff_profile_via_ctypes

        state["hook"] = _ntff_profile_via_ctypes("/opt/axon/libaxon_pjrt.so")
    except Exception:
        state["hook"] = None


# --------------------------------------------------------------------------
def _build():
    import concourse.bass as bass
    import concourse.mybir as mybir
    from concourse.tile import TileContext

    F = mybir.dt.float32
    MD = mybir.dt.float16  # matmul operand dtype
    MULT = mybir.AluOpType.mult
    ADD = mybir.AluOpType.add
    SUB = mybir.AluOpType.subtract
    EXP = mybir.ActivationFunctionType.Exp

    nc = bass.Bass()

    # xTd: host-prearranged [128, 16 nb-blocks, 8 k-chunks, 512 cols] so each
    # (b, nb) projection tile is one contiguous 8KB-per-partition DMA read.
    xTd = nc.dram_tensor("xTd", (128, 16 * 8 * 512), MD, kind="ExternalInput")
    # w*: host-prearranged [128, 8 k-chunks, 128 cols], contiguous per row.
    wq = nc.dram_tensor("wq", (128, 8 * 128), MD, kind="ExternalInput")
    wk = nc.dram_tensor("wk", (128, 8 * 128), MD, kind="ExternalInput")
    wv = nc.dram_tensor("wv", (128, 8 * 128), MD, kind="ExternalInput")
    wo = nc.dram_tensor("wo", (128, C), MD, kind="ExternalInput")
    cosd = nc.dram_tensor("cos", (128, T), MD, kind="ExternalInput")
    sind = nc.dram_tensor("sin2", (128, T), MD, kind="ExternalInput")
    p2d = nc.dram_tensor("p2", (128, 128), MD, kind="ExternalInput")
    bandd = nc.dram_tensor("band01", (128, 256), MD, kind="ExternalInput")
    y = nc.dram_tensor("y", (BT, C), MD, kind="ExternalOutput")
    scr = nc.dram_tensor("scr", (B * 8, 512), F, kind="Internal")

    with TileContext(nc) as tc:
        with (
            tc.tile_pool(name="const", bufs=1) as cst,
            tc.tile_pool(name="xt", bufs=3) as xtp,
            tc.tile_pool(name="qt", bufs=2) as qp,
            tc.tile_pool(name="kt", bufs=2) as kp,
            tc.tile_pool(name="vt", bufs=2) as vp,
            tc.tile_pool(name="ot", bufs=2) as op_,
            tc.tile_pool(name="vst", bufs=2) as vstp,
            tc.tile_pool(name="qs", bufs=2) as qsp,
            tc.tile_pool(name="at", bufs=4) as ap_,
            tc.tile_pool(name="rr", bufs=4) as rrp,
            tc.tile_pool(name="bc", bufs=4) as bcp,
            tc.tile_pool(name="ys", bufs=4) as ysp,
            tc.tile_pool(name="avs", bufs=2) as avsp,
            tc.tile_pool(name="sps", bufs=2, space="PSUM") as sps,
            tc.tile_pool(name="stp", bufs=2, space="PSUM") as stp,
            tc.tile_pool(name="avp", bufs=2, space="PSUM") as avp,
        ):
            # ---- constants (emission order = DMA need order) ---------------
            wq_t = cst.tile([128, 8, 128], MD)
            nc.sync.dma_start(out=wq_t, in_=wq[:, :].rearrange(
                "p (a c) -> p a c", a=8))
            cos_t = cst.tile([128, T], MD)
            nc.sync.dma_start(out=cos_t, in_=cosd[:, :])
            sin_t = cst.tile([128, T], MD)
            nc.sync.dma_start(out=sin_t, in_=sind[:, :])
            wk_t = cst.tile([128, 8, 128], MD)
            nc.sync.dma_start(out=wk_t, in_=wk[:, :].rearrange(
                "p (a c) -> p a c", a=8))
            wv_t = cst.tile([128, 8, 128], MD)
            nc.sync.dma_start(out=wv_t, in_=wv[:, :].rearrange(
                "p (a c) -> p a c", a=8))
            p2_t = cst.tile([128, 128], MD)
            nc.sync.dma_start(out=p2_t, in_=p2d[:, :])
            band_t = cst.tile([128, 256], MD)  # 0/1 mask, [band | band]
            nc.sync.dma_start(out=band_t, in_=bandd[:, :])
            wo_t = cst.tile([128, C], MD)
            nc.sync.dma_start(out=wo_t, in_=wo[:, :])
            band2 = band_t[:, :].rearrange("p (a c) -> p a c", a=2)

            QKV = {}   # b -> (Qb, Kb, Vb)
            xts = {}   # (b, nb) -> xt tile

            def alloc_batch(b):
                Qb = qp.tile([128, T], MD, name="Qb")
                Kb = kp.tile([128, T], MD, name="Kb")
                # per (token-tile, head) an aligned 128-col slot:
                # [d 0..63 | ones | 63 unread] (XBAR transpose needs the
                # 256B-aligned output offsets)
                Vb = vp.tile([128, 16, 2, 128], MD, name="Vb")
                QKV[b] = (Qb, Kb, Vb)
                nc.gpsimd.memset(Vb[:, :, :, 64], 1.0)

            # ---- projection unit: 512 tokens of batch b --------------------
            def xt_item(b, nb):
                def it_xt():
                    # one contiguous 8KB-per-partition read (128 descriptors)
                    xt = xtp.tile([128, 8, 512], MD, name="xt")
                    xts[(b, nb)] = xt
                    g = (b * 4 + nb) * 4096
                    nc.sync.dma_start(
                        out=xt,
                        in_=xTd[:, g : g + 4096].rearrange(
                            "p (a c) -> p a c", a=8))
                return (50, it_xt)

            def proj_items(b, nb):
                """Returns a list of (cost_ns, fn) items; fn emits instrs.
                The unit's xt DMA is NOT included (prefetched separately)."""
                Qb, Kb, Vb = QKV[b]
                cols = slice(nb * 512, (nb + 1) * 512)
                items = []

                state = {}

                def mk_ps(W):
                    def fn():
                        xt = xts[(b, nb)]
                        ps = sps.tile([128, 512], F, tag="p", name="ps")
                        for k in range(8):
                            nc.tensor.matmul(
                                ps[:, :], lhsT=W[:, k, :], rhs=xt[:, k, :],
                                start=(k == 0), stop=(k == 7),
                            )
                        state["ps"] = ps
                    return fn

                def mk_rope_mul(dst):
                    def fn():
                        ps = state["ps"]
                        qs = qsp.tile([128, 512], MD, name="qs")
                        nc.vector.tensor_tensor(qs[:, :], ps[:, :],
                                                sin_t[:, cols], MULT)
                        nc.vector.tensor_tensor(dst[:, cols], ps[:, :],
                                                cos_t[:, cols], MULT)
                        state["qs"] = qs
                    return fn

                def mk_rope_rot(dst):
                    def fn():
                        qs = state.pop("qs")
                        state.pop("ps")
                        rot = sps.tile([128, 512], F, tag="p", name="rot")
                        nc.tensor.matmul(rot[:, :], lhsT=p2_t[:, :],
                                         rhs=qs[:, :], start=True, stop=True)
                        nc.vector.tensor_tensor(dst[:, cols], dst[:, cols],
                                                rot[:, :], SUB)
                    return fn

                items.append((1750, mk_ps(wq_t)))
                items.append((100, mk_rope_mul(Qb)))
                items.append((300, mk_rope_rot(Qb)))
                items.append((1750, mk_ps(wk_t)))
                items.append((100, mk_rope_mul(Kb)))
                items.append((300, mk_rope_rot(Kb)))
                items.append((1750, mk_ps(wv_t)))

                def it_vst():
                    # two partition-0 tiles (the XBAR transpose silently
                    # drops partition-offset sources)
                    ps = state.pop("ps")
                    vst0 = vstp.tile([64, 512], MD, tag="v0", name="vst0")
                    nc.scalar.copy(vst0[:, :], ps[0:64, :])
                    vst1 = vstp.tile([64, 512], MD, tag="v1", name="vst1")
                    nc.scalar.copy(vst1[:, :], ps[64:128, :])
                    state["vst"] = (vst0, vst1)
                items.append((150, it_vst))

                def mk_tr(tl):
                    def fn():
                        # XBAR DMA transpose: [feat 64, tok 128] fp16 ->
                        # token-major, landing per-head in Vb's aligned slots
                        vst0, vst1 = state["vst"]
                        tt = nb * 4 + tl
                        tcs = slice(tl * 128, (tl + 1) * 128)
                        nc.sync.dma_start_transpose(
                            Vb[:, tt, 0, 0:64], vst0[:, tcs])
                        nc.sync.dma_start_transpose(
                            Vb[:, tt, 1, 0:64], vst1[:, tcs])
                    return fn
                for tl in range(4):
                    items.append((60, mk_tr(tl)))
                return items



            # ---- output projection tile (128 tokens) -----------------------
            def mk_ytile(b, Ob, tt):
                def fn():
                    r0 = b * T + tt * 128
                    lhs = Ob[:, tt * 128 : (tt + 1) * 128]
                    ysb = ysp.tile([128, 1024], MD, name="ysb")
                    for nh in (0, 1):
                        yps = sps.tile([128, 512], F, tag="p", name="yps")
                        nc.tensor.matmul(
                            yps[:, :], lhsT=lhs,
                            rhs=wo_t[:, nh * 512 : (nh + 1) * 512],
                            start=True, stop=True,
                        )
                        dst = ysb[:, nh * 512 : (nh + 1) * 512]
                        nc.vector.tensor_copy(dst, yps[:, :])
                    nc.sync.dma_start(out=y[r0 : r0 + 128, :], in_=ysb[:, :])
                return fn

            # ---- filler queue ---------------------------------------------
            fillq = []   # list of (tag, cost, fn)

            def enq(tag, items):
                for cost, fn in items:
                    fillq.append((tag, cost, fn))

            def drain(pred):
                """Emit every queued item whose tag satisfies pred."""
                rest = []
                for tag, cost, fn in fillq:
                    if pred(tag):
                        fn()
                    else:
                        rest.append((tag, cost, fn))
                fillq[:] = rest

            def pop_budget(budget, cur_b):
                # y-tiles of batches >= 2 are reserved to feed the PE during
                # the fillerless last batch
                idx = 0
                while idx < len(fillq) and budget > 0:
                    tag, cost, fn = fillq[idx]
                    if tag[0] == "y" and tag[1] >= 2 and cur_b < 3:
                        idx += 1
                        continue
                    fillq.pop(idx)
                    fn()
                    budget -= cost
                return budget

            # ---- attention for batch b ------------------------------------
            def norm_group(b, g, avs8, Ob):
                """Batched normalize for i-blocks {2g, 2g+1}: one repartition
                DMA + all-lane reciprocal + one writeback, then a stride-0
                DRAM broadcast read and the O multiply per (i, h)."""
                r0 = b * 8 + g * 4
                srt = rrp.tile([128, 16], F, name="srt")
                nc.sync.dma_start(
                    out=srt[:, :], in_=avs8[64:65, 4 * g : 4 * g + 4, :])
                rt = rrp.tile([128, 16], F, name="rt")
                nc.vector.reciprocal(rt[:, :], srt[:, :])
                nc.sync.dma_start(
                    out=scr[r0 : r0 + 4, :].rearrange("a c -> (a c)"),
                    in_=rt[:, :])
                for ih in range(4):
                    i, h = 2 * g + ih // 2, ih % 2
                    bct = bcp.tile([64, 512], F, name="bct")
                    src = scr[r0 + ih : r0 + ih + 1, :]
                    bap = bass.AP(
                        tensor=src.tensor, offset=src.offset,
                        ap=[[0, 64]] + [list(p) for p in src.ap[1:]],
                    )
                    nc.sync.dma_start(out=bct[:, :], in_=bap)
                    nc.vector.tensor_tensor(
                        Ob[64 * h : 64 * h + 64, i * 512 : (i + 1) * 512],
                        avs8[0:64, 4 * g + ih, :], bct[:, :], MULT)

            def phase_d(b):
                Qb, Kb, Vb = QKV[b]
                Ob = op_.tile([128, T], MD, name="Ob")
                avs8 = avsp.tile([65, 8, 512], F, name="avs8")
                for i in range(4):
                    # block i must see its projections emitted already
                    drain(lambda t, b=b, i=i: t[0] == "p" and
                          (t[1], t[2]) <= (b, i))
                    nch = 4 * i + 4
                    avh = [avp.tile([128, 512], F, name="av") for _ in (0, 1)]
                    sts = {}
                    As = {}

                    def emit_qk(j, i=i, sts=sts):
                        delta = j * 128 - i * 512
                        nl = 512 - max(0, delta)
                        off = 512 - nl
                        st = stp.tile([128, 2, 512], F, name="st")
                        for h in (0, 1):
                            hs = slice(64 * h, 64 * h + 64)
                            nc.tensor.matmul(
                                st[:, h, 0:nl],
                                lhsT=Kb[hs, j * 128 : (j + 1) * 128],
                                rhs=Qb[hs, i * 512 + off : (i + 1) * 512],
                                start=True, stop=True,
                            )
                        sts[j] = (st, off, nl, delta >= 0)

                    def emit_exp(j, sts=sts, As=As):
                        st, off, nl, straddle = sts.pop(j)
                        A = ap_.tile([128, 2, 512], MD, name="A")
                        nc.scalar.activation(
                            A[:, :, 0:nl], st[:, :, 0:nl], EXP, scale=SCALE)
                        if straddle:
                            # zero the above-diagonal triangle (fp16 0/1 mask
                            # multiply on the otherwise-idle Pool engine)
                            nc.gpsimd.tensor_tensor(
                                A[:, :, 0:128], A[:, :, 0:128], band2, MULT)
                        As[j] = (A, off, nl)

                    def emit_av(j, nch=nch, As=As, avh=avh):
                        A, off, nl = As.pop(j)
                        for h in (0, 1):
                            nc.tensor.matmul(
                                avh[h][0:65, off:512],
                                lhsT=Vb[:, j, h, 0:65],
                                rhs=A[:, h, 0:nl],
                                start=(j == 0), stop=(j == nch - 1),
                                skip_group_check=True,
                            )

                    for s in range(nch + 2):
                        if s < nch:
                            emit_qk(s)
                        if 1 <= s <= nch:
                            emit_exp(s - 1)
                        if s >= 2:
                            emit_av(s - 2)
                        pop_budget(1600, b)

                    # evacuate O~ + denom row to the SBUF staging tile (frees
                    # the PSUM accumulators immediately; ACT/DVE split)
                    nc.scalar.copy(avs8[:, 2 * i, :], avh[0][0:65, 0:512])
                    nc.vector.tensor_copy(avs8[:, 2 * i + 1, :],
                                          avh[1][0:65, 0:512])

                    if i % 2 == 1:
                        norm_group(b, i // 2, avs8, Ob)
                        # defer the output tiles of both blocks via the queue
                        for tt in range(8 * (i // 2), 8 * (i // 2) + 8):
                            fillq.append(
                                (("y", b, i), 700, mk_ytile(b, Ob, tt)))

            # ---- top-level schedule ---------------------------------------
            def enq_batch(b):
                for nb in range(4):
                    unit = proj_items(b, nb)
                    if nb == 0:
                        unit.insert(0, xt_item(b, 0))
                    if nb + 1 < 4:
                        unit.insert(3, xt_item(b, nb + 1))
                    enq(("p", b, nb), unit)

            alloc_batch(0)
            xt_item(0, 0)[1]()
            xt_item(0, 1)[1]()
            for cost, fn in proj_items(0, 0):
                fn()                      # first unit inline: critical path
            for nb in range(1, 4):
                unit = proj_items(0, nb)
                if nb + 1 < 4:
                    unit.insert(2, xt_item(0, nb + 1))
                enq(("p", 0, nb), unit)
            for b in range(B):
                if b + 1 < B:
                    alloc_batch(b + 1)
                    enq_batch(b + 1)
                phase_d(b)
            drain(lambda t: True)

    _split_sem_waits(nc)
    return nc


# --------------------------------------------------------------------------
def _host_inputs(x, Wq, Wk, Wv, Wo):
    """Per-core input dicts (all shared arrays built once)."""
    BF = np.float16

    # x^T arranged [p, nb-block, k-chunk, col] so each projection tile is one
    # contiguous 8KB-per-partition DMA read
    xT = np.asarray(x, dtype=np.float32).reshape(BT, C).T  # (C, BT)
    xTd = np.ascontiguousarray(
        xT.reshape(8, 128, 16, 512).transpose(1, 2, 0, 3).reshape(128, -1)
    ).astype(BF)

    # NeoX d-permutation within each head: evens then odds
    dperm = np.concatenate([np.arange(0, D, 2), np.arange(1, D, 2)])

    inv_freq = (1.0 / (10000.0 ** (np.arange(0, D, 2) / D))).astype(np.float64)
    pos = np.arange(T, dtype=np.float64)
    ang = pos[None, :] * inv_freq[:, None]  # (32, T)
    cos32 = np.cos(ang).astype(np.float32)
    sin32 = np.sin(ang).astype(np.float32)
    cos_t = np.tile(np.vstack([cos32, cos32]), (2, 1))  # (128, T)
    sin_t = np.tile(np.vstack([-sin32, sin32]), (2, 1))  # (128, T), sign folded

    p2 = np.zeros((128, 128), dtype=np.float32)
    for hb in (0, 64):
        for i2 in range(32):
            p2[hb + i2, hb + 32 + i2] = 1.0
            p2[hb + 32 + i2, hb + i2] = 1.0

    # 0/1 causal mask (1 where q-offset >= k-offset), doubled for head pairs
    band = np.where(
        np.arange(128)[None, :] >= np.arange(128)[:, None], 1.0, 0.0
    ).astype(np.float32)
    band01 = np.concatenate([band, band], axis=1)  # (128, 256)

    def wlayout(w):  # (C, 128) -> (128, 8*128), row-contiguous per k-chunk
        return np.ascontiguousarray(
            w.reshape(8, 128, 128).transpose(1, 0, 2).reshape(128, -1)
        ).astype(BF)

    Wq = np.asarray(Wq, dtype=np.float32)
    Wk = np.asarray(Wk, dtype=np.float32)
    Wv = np.asarray(Wv, dtype=np.float32)
    Wo = np.asarray(Wo, dtype=np.float32)

    in_maps = []
    for c in range(N_CORES):
        sl = slice(128 * c, 128 * (c + 1))
        wq_c = Wq[:, sl].reshape(C, 2, D)[:, :, dperm].reshape(C, 128)
        wk_c = Wk[:, sl].reshape(C, 2, D)[:, :, dperm].reshape(C, 128)
        in_maps.append({
            "xTd": xTd,
            "wq": wlayout(wq_c),
            "wk": wlayout(wk_c),
            "wv": wlayout(Wv[:, sl]),
            "wo": np.ascontiguousarray(Wo[sl, :]).astype(BF),
            "cos": cos_t.astype(BF),
            "sin2": sin_t.astype(BF),
            "p2": p2.astype(BF),
            "band01": band01.astype(BF),
        })
    return in_maps


def kernel(x, Wq, Wk, Wv, Wo, bo):
    global _BUILT, LAST_RESULT
    from concourse.bass_utils import run_bass_kernel_spmd

    if TRACE:
        _install_ntff_hook()

    if _BUILT is None:
        _BUILT = _build()
    nc = _BUILT

    in_maps = _host_inputs(x, Wq, Wk, Wv, Wo)

    last_err = None
    for attempt in range(3):
        try:
            res = run_bass_kernel_spmd(
                nc, in_maps, core_ids=list(range(N_CORES)), trace=TRACE
            )
            break
        except Exception as e:  # transient NRT device errors: retry
            last_err = e
            import time as _time

            _time.sleep(2.0)
    else:
        raise last_err
    LAST_RESULT = res

    acc = res.results[0]["y"].astype(np.float64)
    for c in range(1, N_CORES):
        acc = acc + res.results[c]["y"]
    out = acc.astype(np.float32) + np.asarray(bo, dtype=np.float32)[None, :]
    return out.reshape(B, T, C)


# revision 31
# speedup vs baseline: 1.3816x; 1.1831x over previous
"""Causal multi-head self-attention (RoPE) Trainium2 Bass kernel.

Problem: x:(4,2048,1024), Wq/Wk/Wv:(1024,1024), Wo:(1024,1024), bo:(1024,)
  q,k,v = split_heads(x@W*), rope(q), rope(k), causal softmax(q k^T/8) v, @Wo+bo

Sharding: head-parallel across 8 cores. Core c owns heads {2c, 2c+1} for all
4 batches: it computes q/k/v projections against the 128-column weight slice,
attention for its heads, and a partial output projection against the matching
128-row slice of Wo. Host sums the 8 partial (8192,1024) outputs and adds bo.

On-core layout (all "T" tensors are feature-major: partitions=feature rows,
free=tokens):
  Q^T/K^T (128 x 2048/batch): rows = [h0 d-evens(32), h0 d-odds(32), h1 ...]
    (NeoX-style d-permutation, folded into the host-permuted weight columns;
     valid because q and k get the same permutation and qk^T is d-invariant)
  RoPE: Q <- Q*cos + (P2@Q)*sin2, where P2 swaps the even/odd halves per head
    (PE matmul) and sin2 carries the sign; 3 DVE passes per tensor-block.
  S^T tiles (tj x ti) = K^T.T @ Q^T per head (fp32r, K=64 contraction).
  A = exp(0.125*S^T) (ACT, straddle tiles band-masked with -1e30 triangle).
  O~^T (65 x ti) accumulated = [V|1].T @ A over tj chunks; row 64 = softmax
    denominators (ones column trick). Normalize via ACT reciprocal +
    DRAM-staged partition broadcast + DVE multiply -> O^T (128 x 2048).
  y partial (128t x 1024) fp16 = O^T-chunk.T @ Wo-slice, via SBUF staging.
"""

import numpy as np

B, T, C = 4, 2048, 1024
H, D = 16, 64
N_CORES = 8
BT = B * T
SCALE = 0.125  # D**-0.5
NEG = -1.0e30

TRACE = False            # set True (e.g. from test.py) to capture an NTFF trace
LAST_RESULT = None       # BassKernelResults of the most recent run

_BUILT = None            # cached (nc, input-name list)


# --------------------------------------------------------------------------
# workaround: this walrus build rejects >1 semaphore wait per instruction
def _split_sem_waits(nc, max_waits=1):
    import concourse.mybir as mybir

    n = 0
    for f in nc.m.functions:
        for bb in f.blocks:
            insts = bb.instructions
            idx = 0
            while idx < len(insts):
                i = insts[idx]
                si = getattr(i, "sync_info", None)
                if si is not None and si.on_wait and len(si.on_wait) > max_waits:
                    waits = list(si.on_wait)
                    extra, keep = waits[:-max_waits], waits[-max_waits:]
                    si.on_wait = keep
                    pos = idx
                    for j in range(0, len(extra), max_waits):
                        n += 1
                        nd = mybir.InstNoOp(name=f"I-waitsplit-{n}", ins=[], outs=[])
                        nd.engine = i.engine
                        nd.sync_info = mybir.SyncInfo(
                            on_wait=extra[j : j + max_waits], on_update=[]
                        )
                        insts.insert(pos, nd)
                        pos += 1
                    idx = pos
                idx += 1


def _install_ntff_hook():
    """The image's antenv lacks axon_hooks; synthesize it so trace=True works."""
    import sys
    import types

    if "antenv.axon_hooks" in sys.modules:
        return
    import antenv

    state = {"hook": None}
    mod = types.ModuleType("antenv.axon_hooks")
    mod.get_axon_ntff_profile_hook = lambda: state["hook"]
    mod.set_axon_ntff_profile_hook = lambda h: state.__setitem__("hook", h)
    sys.modules["antenv.axon_hooks"] = mod
    antenv.axon_hooks = mod
    try:
        from trn_agent_boot.trn_boot import _ntff_profile_via_ctypes

        state["hook"] = _ntff_profile_via_ctypes("/opt/axon/libaxon_pjrt.so")
    except Exception:
        state["hook"] = None


# --------------------------------------------------------------------------
def _build():
    import concourse.bass as bass
    import concourse.mybir as mybir
    from concourse.tile import TileContext

    F = mybir.dt.float32
    MD = mybir.dt.float16  # matmul operand dtype
    MULT = mybir.AluOpType.mult
    ADD = mybir.AluOpType.add
    SUB = mybir.AluOpType.subtract
    EXP = mybir.ActivationFunctionType.Exp

    nc = bass.Bass()

    xT = nc.dram_tensor("xT", (C, BT), MD, kind="ExternalInput")
    wq = nc.dram_tensor("wq", (C, 128), MD, kind="ExternalInput")
    wk = nc.dram_tensor("wk", (C, 128), MD, kind="ExternalInput")
    wv = nc.dram_tensor("wv", (C, 128), MD, kind="ExternalInput")
    wo = nc.dram_tensor("wo", (128, C), MD, kind="ExternalInput")
    cosd = nc.dram_tensor("cos", (128, T), MD, kind="ExternalInput")
    sind = nc.dram_tensor("sin2", (128, T), MD, kind="ExternalInput")
    p2d = nc.dram_tensor("p2", (128, 128), MD, kind="ExternalInput")
    bandd = nc.dram_tensor("band2x", (128, 256), F, kind="ExternalInput")
    id2d = nc.dram_tensor("id2", (128, 64), F, kind="ExternalInput")
    y = nc.dram_tensor("y", (BT, C), MD, kind="ExternalOutput")
    scr = nc.dram_tensor("scr", (B * 8, 512), F, kind="Internal")

    with TileContext(nc) as tc:
        with (
            tc.tile_pool(name="const", bufs=1) as cst,
            tc.tile_pool(name="xt", bufs=3) as xtp,
            tc.tile_pool(name="qt", bufs=2) as qp,
            tc.tile_pool(name="kt", bufs=2) as kp,
            tc.tile_pool(name="vt", bufs=2) as vp,
            tc.tile_pool(name="ot", bufs=2) as op_,
            tc.tile_pool(name="vst", bufs=2) as vstp,
            tc.tile_pool(name="tmp", bufs=4) as tmp,
            tc.tile_pool(name="at", bufs=6) as ap_,
            tc.tile_pool(name="bc", bufs=4) as bcp,
            tc.tile_pool(name="avs", bufs=4) as avsp,
            tc.tile_pool(name="rr", bufs=4) as rp,
            tc.tile_pool(name="ys", bufs=4) as ysp,
            tc.tile_pool(name="sps", bufs=2, space="PSUM") as sps,
            tc.tile_pool(name="stp", bufs=2, space="PSUM") as stp,
            tc.tile_pool(name="avp", bufs=2, space="PSUM") as avp,
        ):
            # ---- constants -------------------------------------------------
            wq_t = cst.tile([128, 8, 128], MD)
            wk_t = cst.tile([128, 8, 128], MD)
            wv_t = cst.tile([128, 8, 128], MD)
            for k in range(8):
                nc.sync.dma_start(out=wq_t[:, k, :], in_=wq[k * 128 : (k + 1) * 128, :])
                nc.sync.dma_start(out=wk_t[:, k, :], in_=wk[k * 128 : (k + 1) * 128, :])
                nc.sync.dma_start(out=wv_t[:, k, :], in_=wv[k * 128 : (k + 1) * 128, :])
            wo_t = cst.tile([128, C], MD)
            nc.sync.dma_start(out=wo_t, in_=wo[:, :])
            cos_t = cst.tile([128, T], MD)
            nc.sync.dma_start(out=cos_t, in_=cosd[:, :])
            sin_t = cst.tile([128, T], MD)
            nc.sync.dma_start(out=sin_t, in_=sind[:, :])
            p2_t = cst.tile([128, 128], MD)
            nc.sync.dma_start(out=p2_t, in_=p2d[:, :])
            band_t = cst.tile([128, 256], F)  # [band | band] for head pairs
            nc.sync.dma_start(out=band_t, in_=bandd[:, :])
            id_t = cst.tile([128, 64], F)
            nc.sync.dma_start(out=id_t, in_=id2d[:, :])

            QKV = {}  # b -> (Qb, Kb, Vb);  O = {} b -> Ob

            def phase_a_alloc(b):
                Qb = qp.tile([128, T], MD, name="Qb")
                Kb = kp.tile([128, T], MD, name="Kb")
                Vb = vp.tile([128, 16, 256], MD, name="Vb")  # per head 128 cols:
                # [d 0..63 | ones | zeros*63] so the AV lhsT is 128-wide (FWL)
                QKV[b] = (Qb, Kb, Vb)
                # ones + zero padding via memsets on the idle Pool engine
                # (DRAM-sourced fills cost thousands of tiny DMA descriptors)
                vv = Vb[:, :, :].rearrange("p t (a c) -> p t a c", a=2)
                nc.gpsimd.memset(vv[:, :, :, 64], 1.0)
                nc.gpsimd.memset(vv[:, :, :, 65:128], 0.0)

            def phase_a_unit(b, nb):
                Qb, Kb, Vb = QKV[b]
                if True:
                    g0 = b * T + nb * 512
                    cols = slice(nb * 512, (nb + 1) * 512)
                    xt = xtp.tile([128, 8, 512], MD, name="xt")
                    for k in range(8):
                        nc.sync.dma_start(
                            out=xt[:, k, :],
                            in_=xT[k * 128 : (k + 1) * 128, g0 : g0 + 512],
                        )
                    for W, dst in ((wq_t, Qb), (wk_t, Kb)):
                        ps = sps.tile([128, 512], F, tag="s", name="ps")
                        for k in range(8):
                            nc.tensor.matmul(
                                ps[:, :], lhsT=W[:, k, :], rhs=xt[:, k, :],
                                start=(k == 0), stop=(k == 7),
                            )
                        # rope: dst = qr*cos - P2@(qr*sin2)
                        #   (P2@ (q.sin2))[p] = -q~[p]*sin2[p], since sin2 is
                        #    antisymmetric and cos symmetric under the pair swap
                        qr = tmp.tile([128, 512], MD, name="qr")
                        nc.scalar.copy(qr[:, :], ps[:, :])
                        qs = tmp.tile([128, 512], MD, name="qs")
                        nc.vector.tensor_tensor(qs[:, :], qr[:, :],
                                                sin_t[:, cols], MULT)
                        nc.vector.tensor_tensor(dst[:, cols], qr[:, :],
                                                cos_t[:, cols], MULT)
                        rot = sps.tile([128, 512], F, tag="s", name="rot")
                        nc.tensor.matmul(rot[:, :], lhsT=p2_t[:, :], rhs=qs[:, :],
                                         start=True, stop=True)
                        nc.vector.tensor_tensor(dst[:, cols], dst[:, cols],
                                                rot[:, :], SUB)
                    ps = sps.tile([128, 512], F, tag="s", name="ps")
                    for k in range(8):
                        nc.tensor.matmul(
                            ps[:, :], lhsT=wv_t[:, k, :], rhs=xt[:, k, :],
                            start=(k == 0), stop=(k == 7),
                        )
                    vst = vstp.tile([128, 512], F, name="vst")
                    nc.scalar.copy(vst[:, :], ps[:, :])
                    for tl in range(4):
                        tt = nb * 4 + tl
                        tcs = slice(tl * 128, (tl + 1) * 128)
                        for h in (0, 1):
                            tp = sps.tile([128, 64], F, tag="s", name="tp")
                            nc.tensor.transpose(
                                tp[:, :], vst[64 * h : 64 * h + 64, tcs],
                                id_t[64 * h : 64 * h + 64, :],
                            )
                            nc.vector.tensor_copy(
                                Vb[:, tt, 128 * h : 128 * h + 64], tp[:, :])

            def y_unit(b, Ob, i):
                # output projection for the 4 token-tiles of ti-block i
                for tt in range(4 * i, 4 * i + 4):
                    lhs = Ob[:, tt * 128 : (tt + 1) * 128]
                    ysb = ysp.tile([128, 1024], MD, name="ysb")
                    for nh in (0, 1):
                        yps = sps.tile([128, 512], F, tag="s", name="yps")
                        nc.tensor.matmul(
                            yps[:, :], lhsT=lhs,
                            rhs=wo_t[:, nh * 512 : (nh + 1) * 512],
                            start=True, stop=True,
                        )
                        if nh == 0:
                            nc.vector.tensor_copy(ysb[:, 0:512], yps[:, :])
                        else:
                            nc.scalar.copy(ysb[:, 512:1024], yps[:, :])
                    r0 = b * T + tt * 128
                    nc.sync.dma_start(out=y[r0 : r0 + 128, :], in_=ysb[:, :])

            def phase_d(b, filler=None, pre=None):
                Qb, Kb, Vb = QKV[b]
                Ob = op_.tile([128, T], MD, name="Ob")
                pending = []  # deferred y_units: keep normalize latency off
                # the PE critical path by emitting them a ti-block later
                for i in range(4):
                    if pre is not None:
                        pre(i)
                    av = [avp.tile([128, 512], F, tag="av", name="av")
                          for _ in (0, 1)]
                    nch = 4 * i + 4
                    sts = {}

                    def emit_st(j):
                        delta = j * 128 - i * 512
                        nl = 512 - max(0, delta)
                        off = 512 - nl
                        st = stp.tile([128, 2, 512], F, name="st")
                        for h in (0, 1):
                            hs = slice(64 * h, 64 * h + 64)
                            nc.tensor.matmul(
                                st[:, h, 0:nl],
                                lhsT=Kb[hs, j * 128 : (j + 1) * 128],
                                rhs=Qb[hs, i * 512 + off : (i + 1) * 512],
                                start=True, stop=True,
                            )
                        if delta >= 0:  # straddles the diagonal: mask triangle
                            nc.vector.tensor_tensor(
                                st[:, :, 0:128], st[:, :, 0:128],
                                band_t[:, :].rearrange("p (a c) -> p a c", a=2),
                                ADD)
                        sts[j] = (st, off, nl)

                    LAG = 1
                    for j in range(min(LAG, nch)):
                        emit_st(j)
                    for j in range(nch):
                        if j + LAG < nch:
                            emit_st(j + LAG)
                        if j == 1 and pending:
                            y_unit(b, Ob, pending.pop(0))
                        st, off, nl = sts.pop(j)
                        A = ap_.tile([128, 2, 512], MD, name="A")
                        nc.scalar.activation(
                            A[:, :, 0:nl], st[:, :, 0:nl], EXP, scale=SCALE)
                        for h in (0, 1):
                            nc.tensor.matmul(
                                av[h][0:128, off:512],
                                lhsT=Vb[:, j, 128 * h : 128 * h + 128],
                                rhs=A[:, h, 0:nl],
                                start=(j == 0), stop=(j == nch - 1),
                                skip_group_check=True,
                            )
                    for h in (0, 1):
                        row = b * 8 + i * 2 + h
                        # evacuate the accumulator to SBUF at once so the
                        # PSUM slot recycles without waiting on the
                        # reciprocal/broadcast DMA chain
                        avs = avsp.tile([65, 512], F, name="avs")
                        nc.vector.tensor_copy(avs[:, :], av[h][0:65, :])
                        # sums row -> DRAM -> (128x4) repartition -> lane-
                        # parallel reciprocal -> DRAM -> 64-row broadcast
                        srt = rp.tile([128, 4], F, name="srt")
                        nc.sync.dma_start(out=srt[:, :], in_=avs[64:65, :])
                        rt = rp.tile([128, 4], F, name="rt")
                        nc.vector.reciprocal(rt[:, :], srt[:, :])
                        nc.sync.dma_start(
                            out=scr[row : row + 1, :].rearrange(
                                "r (p c) -> (r p) c", c=4),
                            in_=rt[:, :],
                        )
                        bct = bcp.tile([64, 512], F, name="bct")
                        src = scr[row : row + 1, :]
                        bap = bass.AP(
                            tensor=src.tensor, offset=src.offset,
                            ap=[[0, 64]] + [list(p) for p in src.ap[1:]],
                        )
                        nc.sync.dma_start(out=bct[:, :], in_=bap)
                        nc.vector.tensor_tensor(
                            Ob[64 * h : 64 * h + 64, i * 512 : (i + 1) * 512],
                            avs[0:64, :], bct[:, :], MULT,
                        )
                    pending.append(i)
                    if filler is not None:
                        filler(i)
                for i2 in pending:
                    y_unit(b, Ob, i2)

            phase_a_alloc(0)
            for b in range(B):
                if b + 1 < B:
                    phase_a_alloc(b + 1)
                    fil = (lambda i, nb=b + 1: phase_a_unit(nb, i))
                else:
                    fil = None
                # batch 0's projection blocks are emitted just-in-time ahead
                # of the attention block that first needs them
                pre = (lambda i: phase_a_unit(0, i)) if b == 0 else None
                phase_d(b, filler=fil, pre=pre)

    _split_sem_waits(nc)
    return nc


# --------------------------------------------------------------------------
def _host_inputs(x, Wq, Wk, Wv):
    """Per-core input dicts (all shared arrays built once)."""
    BF = np.float16
    xT = np.ascontiguousarray(
        np.asarray(x, dtype=np.float32).reshape(BT, C).T).astype(BF)

    # NeoX d-permutation within each head: evens then odds
    dperm = np.concatenate([np.arange(0, D, 2), np.arange(1, D, 2)])

    inv_freq = (1.0 / (10000.0 ** (np.arange(0, D, 2) / D))).astype(np.float64)
    pos = np.arange(T, dtype=np.float64)
    ang = pos[None, :] * inv_freq[:, None]  # (32, T)
    cos32 = np.cos(ang).astype(np.float32)
    sin32 = np.sin(ang).astype(np.float32)
    cos_t = np.tile(np.vstack([cos32, cos32]), (2, 1))  # (128, T)
    sin_t = np.tile(np.vstack([-sin32, sin32]), (2, 1))  # (128, T), sign folded

    p2 = np.zeros((128, 128), dtype=np.float32)
    for hb in (0, 64):
        for i2 in range(32):
            p2[hb + i2, hb + 32 + i2] = 1.0
            p2[hb + 32 + i2, hb + i2] = 1.0

    band = np.where(
        np.arange(128)[None, :] < np.arange(128)[:, None], np.float32(NEG), 0.0
    ).astype(np.float32)
    band2x = np.concatenate([band, band], axis=1)  # (128, 256)
    id2 = np.tile(np.eye(D, dtype=np.float32), (2, 1))  # (128, 64)

    Wq = np.asarray(Wq, dtype=np.float32)
    Wk = np.asarray(Wk, dtype=np.float32)
    Wv = np.asarray(Wv, dtype=np.float32)

    in_maps = []
    for c in range(N_CORES):
        sl = slice(128 * c, 128 * (c + 1))
        wq_c = Wq[:, sl].reshape(C, 2, D)[:, :, dperm].reshape(C, 128)
        wk_c = Wk[:, sl].reshape(C, 2, D)[:, :, dperm].reshape(C, 128)
        in_maps.append({
            "xT": xT,
            "wq": np.ascontiguousarray(wq_c).astype(BF),
            "wk": np.ascontiguousarray(wk_c).astype(BF),
            "wv": np.ascontiguousarray(Wv[:, sl]).astype(BF),
            "wo": None,  # set below
            "cos": cos_t.astype(BF),
            "sin2": sin_t.astype(BF),
            "p2": p2.astype(BF),
            "band2x": band2x,
            "id2": id2,
        })
    return in_maps


def kernel(x, Wq, Wk, Wv, Wo, bo):
    global _BUILT, LAST_RESULT
    from concourse.bass_utils import run_bass_kernel_spmd

    if TRACE:
        _install_ntff_hook()

    if _BUILT is None:
        _BUILT = _build()
    nc = _BUILT

    in_maps = _host_inputs(x, Wq, Wk, Wv)
    Wo = np.asarray(Wo, dtype=np.float32)
    for c in range(N_CORES):
        in_maps[c]["wo"] = np.ascontiguousarray(
            Wo[128 * c : 128 * (c + 1), :]).astype(np.float16)

    last_err = None
    for attempt in range(3):
        try:
            res = run_bass_kernel_spmd(
                nc, in_maps, core_ids=list(range(N_CORES)), trace=TRACE
            )
            break
        except Exception as e:  # transient NRT device errors: retry
            last_err = e
            import time as _time

            _time.sleep(2.0)
    else:
        raise last_err
    LAST_RESULT = res

    acc = res.results[0]["y"].astype(np.float64)
    for c in range(1, N_CORES):
        acc = acc + res.results[c]["y"]
    out = acc.astype(np.float32) + np.asarray(bo, dtype=np.float32)[None, :]
    return out.reshape(B, T, C)


# revision 33
# speedup vs baseline: 1.4364x; 1.0397x over previous
"""Causal multi-head self-attention (RoPE) Trainium2 Bass kernel.

Problem: x:(4,2048,1024), Wq/Wk/Wv:(1024,1024), Wo:(1024,1024), bo:(1024,)
  q,k,v = split_heads(x@W*), rope(q), rope(k), causal softmax(q k^T/8) v, @Wo+bo

Sharding: head-parallel across 8 cores. Core c owns heads {2c, 2c+1} for all
4 batches: it computes q/k/v projections against the 128-column weight slice,
attention for its heads, and a partial output projection against the matching
128-row slice of Wo. Host sums the 8 partial (8192,1024) outputs and adds bo.

On-core layout (all "T" tensors are feature-major: partitions=feature rows,
free=tokens):
  Q^T/K^T (128 x 2048/batch): rows = [h0 d-evens(32), h0 d-odds(32), h1 ...]
    (NeoX-style d-permutation, folded into the host-permuted weight columns;
     valid because q and k get the same permutation and qk^T is d-invariant)
  RoPE: Q <- Q*cos + (P2@Q)*sin2, where P2 swaps the even/odd halves per head
    (PE matmul) and sin2 carries the sign; 3 DVE passes per tensor-block.
  S^T tiles (tj x ti) = K^T.T @ Q^T per head (fp32r, K=64 contraction).
  A = exp(0.125*S^T) (ACT, straddle tiles band-masked with -1e30 triangle).
  O~^T (65 x ti) accumulated = [V|1].T @ A over tj chunks; row 64 = softmax
    denominators (ones column trick). Normalize via ACT reciprocal +
    DRAM-staged partition broadcast + DVE multiply -> O^T (128 x 2048).
  y partial (128t x 1024) fp16 = O^T-chunk.T @ Wo-slice, via SBUF staging.
"""

import numpy as np

B, T, C = 4, 2048, 1024
H, D = 16, 64
N_CORES = 8
BT = B * T
SCALE = 0.125  # D**-0.5
NEG = -1.0e30

TRACE = False            # set True (e.g. from test.py) to capture an NTFF trace
LAST_RESULT = None       # BassKernelResults of the most recent run

_BUILT = None            # cached (nc, input-name list)


# --------------------------------------------------------------------------
# workaround: this walrus build rejects >1 semaphore wait per instruction
def _split_sem_waits(nc, max_waits=1):
    import concourse.mybir as mybir

    n = 0
    for f in nc.m.functions:
        for bb in f.blocks:
            insts = bb.instructions
            idx = 0
            while idx < len(insts):
                i = insts[idx]
                si = getattr(i, "sync_info", None)
                if si is not None and si.on_wait and len(si.on_wait) > max_waits:
                    waits = list(si.on_wait)
                    extra, keep = waits[:-max_waits], waits[-max_waits:]
                    si.on_wait = keep
                    pos = idx
                    for j in range(0, len(extra), max_waits):
                        n += 1
                        nd = mybir.InstNoOp(name=f"I-waitsplit-{n}", ins=[], outs=[])
                        nd.engine = i.engine
                        nd.sync_info = mybir.SyncInfo(
                            on_wait=extra[j : j + max_waits], on_update=[]
                        )
                        insts.insert(pos, nd)
                        pos += 1
                    idx = pos
                idx += 1


def _install_ntff_hook():
    """The image's antenv lacks axon_hooks; synthesize it so trace=True works."""
    import sys
    import types

    if "antenv.axon_hooks" in sys.modules:
        return
    import antenv

    state = {"hook": None}
    mod = types.ModuleType("antenv.axon_hooks")
    mod.get_axon_ntff_profile_hook = lambda: state["hook"]
    mod.set_axon_ntff_profile_hook = lambda h: state.__setitem__("hook", h)
    sys.modules["antenv.axon_hooks"] = mod
    antenv.axon_hooks = mod
    try:
        from trn_agent_boot.trn_boot import _ntff_profile_via_ctypes

        state["hook"] = _ntff_profile_via_ctypes("/opt/axon/libaxon_pjrt.so")
    except Exception:
        state["hook"] = None


# --------------------------------------------------------------------------
def _build():
    import concourse.bass as bass
    import concourse.mybir as mybir
    from concourse.tile import TileContext

    F = mybir.dt.float32
    MD = mybir.dt.float16  # matmul operand dtype
    MULT = mybir.AluOpType.mult
    ADD = mybir.AluOpType.add
    SUB = mybir.AluOpType.subtract
    EXP = mybir.ActivationFunctionType.Exp

    nc = bass.Bass()

    xT = nc.dram_tensor("xT", (C, BT), MD, kind="ExternalInput")
    wq = nc.dram_tensor("wq", (C, 128), MD, kind="ExternalInput")
    wk = nc.dram_tensor("wk", (C, 128), MD, kind="ExternalInput")
    wv = nc.dram_tensor("wv", (C, 128), MD, kind="ExternalInput")
    wo = nc.dram_tensor("wo", (128, C), MD, kind="ExternalInput")
    cosd = nc.dram_tensor("cos", (128, T), MD, kind="ExternalInput")
    sind = nc.dram_tensor("sin2", (128, T), MD, kind="ExternalInput")
    p2d = nc.dram_tensor("p2", (128, 128), MD, kind="ExternalInput")
    bandd = nc.dram_tensor("band2x", (128, 256), F, kind="ExternalInput")
    id2d = nc.dram_tensor("id2", (128, 64), F, kind="ExternalInput")
    y = nc.dram_tensor("y", (BT, C), MD, kind="ExternalOutput")
    scr = nc.dram_tensor("scr", (B * 8, 512), F, kind="Internal")

    with TileContext(nc) as tc:
        with (
            tc.tile_pool(name="const", bufs=1) as cst,
            tc.tile_pool(name="xt", bufs=3) as xtp,
            tc.tile_pool(name="qt", bufs=2) as qp,
            tc.tile_pool(name="kt", bufs=2) as kp,
            tc.tile_pool(name="vt", bufs=2) as vp,
            tc.tile_pool(name="ot", bufs=2) as op_,
            tc.tile_pool(name="vst", bufs=2) as vstp,
            tc.tile_pool(name="tmp", bufs=4) as tmp,
            tc.tile_pool(name="at", bufs=6) as ap_,
            tc.tile_pool(name="bc", bufs=4) as bcp,
            tc.tile_pool(name="avs", bufs=4) as avsp,
            tc.tile_pool(name="rr", bufs=4) as rp,
            tc.tile_pool(name="ys", bufs=4) as ysp,
            tc.tile_pool(name="sps", bufs=2, space="PSUM") as sps,
            tc.tile_pool(name="stp", bufs=2, space="PSUM") as stp,
            tc.tile_pool(name="avp", bufs=2, space="PSUM") as avp,
        ):
            # ---- constants -------------------------------------------------
            wq_t = cst.tile([128, 8, 128], MD)
            wk_t = cst.tile([128, 8, 128], MD)
            wv_t = cst.tile([128, 8, 128], MD)
            for k in range(8):
                nc.sync.dma_start(out=wq_t[:, k, :], in_=wq[k * 128 : (k + 1) * 128, :])
                nc.sync.dma_start(out=wk_t[:, k, :], in_=wk[k * 128 : (k + 1) * 128, :])
                nc.sync.dma_start(out=wv_t[:, k, :], in_=wv[k * 128 : (k + 1) * 128, :])
            wo_t = cst.tile([128, C], MD)
            nc.sync.dma_start(out=wo_t, in_=wo[:, :])
            cos_t = cst.tile([128, T], MD)
            nc.sync.dma_start(out=cos_t, in_=cosd[:, :])
            sin_t = cst.tile([128, T], MD)
            nc.sync.dma_start(out=sin_t, in_=sind[:, :])
            p2_t = cst.tile([128, 128], MD)
            nc.sync.dma_start(out=p2_t, in_=p2d[:, :])
            band_t = cst.tile([128, 256], F)  # [band | band] for head pairs
            nc.sync.dma_start(out=band_t, in_=bandd[:, :])
            id_t = cst.tile([128, 64], F)
            nc.sync.dma_start(out=id_t, in_=id2d[:, :])

            QKV = {}  # b -> (Qb, Kb, Vb);  O = {} b -> Ob

            def phase_a_alloc(b):
                Qb = qp.tile([128, T], MD, name="Qb")
                Kb = kp.tile([128, T], MD, name="Kb")
                Vb = vp.tile([128, 16, 256], MD, name="Vb")  # per head 128 cols:
                # [d 0..63 | ones | zeros*63] so the AV lhsT is 128-wide (FWL)
                QKV[b] = (Qb, Kb, Vb)
                # ones + zero padding via memsets on the idle Pool engine
                # (DRAM-sourced fills cost thousands of tiny DMA descriptors)
                vv = Vb[:, :, :].rearrange("p t (a c) -> p t a c", a=2)
                nc.gpsimd.memset(vv[:, :, :, 64], 1.0)
                nc.gpsimd.memset(vv[:, :, :, 65:128], 0.0)

            def phase_a_unit(b, nb):
                Qb, Kb, Vb = QKV[b]
                if True:
                    g0 = b * T + nb * 512
                    cols = slice(nb * 512, (nb + 1) * 512)
                    xt = xtp.tile([128, 8, 512], MD, name="xt")
                    for k in range(8):
                        nc.sync.dma_start(
                            out=xt[:, k, :],
                            in_=xT[k * 128 : (k + 1) * 128, g0 : g0 + 512],
                        )
                    for W, dst in ((wq_t, Qb), (wk_t, Kb)):
                        ps = sps.tile([128, 512], F, tag="s", name="ps")
                        for k in range(8):
                            nc.tensor.matmul(
                                ps[:, :], lhsT=W[:, k, :], rhs=xt[:, k, :],
                                start=(k == 0), stop=(k == 7),
                            )
                        # rope: dst = qr*cos - P2@(qr*sin2)
                        #   (P2@ (q.sin2))[p] = -q~[p]*sin2[p], since sin2 is
                        #    antisymmetric and cos symmetric under the pair swap
                        qr = tmp.tile([128, 512], MD, name="qr")
                        nc.scalar.copy(qr[:, :], ps[:, :])
                        qs = tmp.tile([128, 512], MD, name="qs")
                        nc.vector.tensor_tensor(qs[:, :], qr[:, :],
                                                sin_t[:, cols], MULT)
                        nc.vector.tensor_tensor(dst[:, cols], qr[:, :],
                                                cos_t[:, cols], MULT)
                        rot = sps.tile([128, 512], F, tag="s", name="rot")
                        nc.tensor.matmul(rot[:, :], lhsT=p2_t[:, :], rhs=qs[:, :],
                                         start=True, stop=True)
                        nc.vector.tensor_tensor(dst[:, cols], dst[:, cols],
                                                rot[:, :], SUB)
                    ps = sps.tile([128, 512], F, tag="s", name="ps")
                    for k in range(8):
                        nc.tensor.matmul(
                            ps[:, :], lhsT=wv_t[:, k, :], rhs=xt[:, k, :],
                            start=(k == 0), stop=(k == 7),
                        )
                    vst = vstp.tile([128, 512], F, name="vst")
                    nc.scalar.copy(vst[:, :], ps[:, :])
                    for tl in range(4):
                        tt = nb * 4 + tl
                        tcs = slice(tl * 128, (tl + 1) * 128)
                        for h in (0, 1):
                            tp = sps.tile([128, 64], F, tag="s", name="tp")
                            nc.tensor.transpose(
                                tp[:, :], vst[64 * h : 64 * h + 64, tcs],
                                id_t[64 * h : 64 * h + 64, :],
                            )
                            nc.vector.tensor_copy(
                                Vb[:, tt, 128 * h : 128 * h + 64], tp[:, :])

            def y_unit(b, Ob, i):
                # output projection for the 4 token-tiles of ti-block i
                for tt in range(4 * i, 4 * i + 4):
                    lhs = Ob[:, tt * 128 : (tt + 1) * 128]
                    ysb = ysp.tile([128, 1024], MD, name="ysb")
                    for nh in (0, 1):
                        yps = sps.tile([128, 512], F, tag="s", name="yps")
                        nc.tensor.matmul(
                            yps[:, :], lhsT=lhs,
                            rhs=wo_t[:, nh * 512 : (nh + 1) * 512],
                            start=True, stop=True,
                        )
                        if nh == 0:
                            nc.vector.tensor_copy(ysb[:, 0:512], yps[:, :])
                        else:
                            nc.scalar.copy(ysb[:, 512:1024], yps[:, :])
                    r0 = b * T + tt * 128
                    nc.sync.dma_start(out=y[r0 : r0 + 128, :], in_=ysb[:, :])

            def phase_d(b, filler=None, pre=None):
                Qb, Kb, Vb = QKV[b]
                Ob = op_.tile([128, T], MD, name="Ob")
                pending = []  # deferred y_units: keep normalize latency off
                # the PE critical path by emitting them a ti-block later
                for i in range(4):
                    if pre is not None:
                        pre(i)
                    av = [avp.tile([128, 512], F, tag="av", name="av")
                          for _ in (0, 1)]
                    nch = 4 * i + 4
                    sts = {}

                    def emit_st(j):
                        delta = j * 128 - i * 512
                        nl = 512 - max(0, delta)
                        off = 512 - nl
                        st = stp.tile([128, 2, 512], F, name="st")
                        for h in (0, 1):
                            hs = slice(64 * h, 64 * h + 64)
                            nc.tensor.matmul(
                                st[:, h, 0:nl],
                                lhsT=Kb[hs, j * 128 : (j + 1) * 128],
                                rhs=Qb[hs, i * 512 + off : (i + 1) * 512],
                                start=True, stop=True,
                            )
                        if delta >= 0:  # straddles the diagonal: mask triangle
                            nc.vector.tensor_tensor(
                                st[:, :, 0:128], st[:, :, 0:128],
                                band_t[:, :].rearrange("p (a c) -> p a c", a=2),
                                ADD)
                        sts[j] = (st, off, nl)

                    LAG = 1
                    for j in range(min(LAG, nch)):
                        emit_st(j)
                    for j in range(nch):
                        if j + LAG < nch:
                            emit_st(j + LAG)
                        if j in (1, 3) and pending:
                            y_unit(b, Ob, pending.pop(0))
                        st, off, nl = sts.pop(j)
                        A = ap_.tile([128, 2, 512], MD, name="A")
                        nc.scalar.activation(
                            A[:, :, 0:nl], st[:, :, 0:nl], EXP, scale=SCALE)
                        for h in (0, 1):
                            nc.tensor.matmul(
                                av[h][0:128, off:512],
                                lhsT=Vb[:, j, 128 * h : 128 * h + 128],
                                rhs=A[:, h, 0:nl],
                                start=(j == 0), stop=(j == nch - 1),
                                skip_group_check=True,
                            )
                    for h in (0, 1):
                        row = b * 8 + i * 2 + h
                        # evacuate the accumulator to SBUF at once so the
                        # PSUM slot recycles without waiting on the
                        # reciprocal/broadcast DMA chain
                        # split across ACT/DVE so both accumulator banks
                        # recycle fast (av pool has no cross-block slack)
                        avs = avsp.tile([65, 512], F, name="avs")
                        if h == 0:
                            nc.scalar.copy(avs[:, :], av[h][0:65, :])
                        else:
                            nc.vector.tensor_copy(avs[:, :], av[h][0:65, :])
                        # sums row -> DRAM -> (128x4) repartition -> lane-
                        # parallel reciprocal -> DRAM -> 64-row broadcast
                        srt = rp.tile([128, 4], F, name="srt")
                        nc.sync.dma_start(out=srt[:, :], in_=avs[64:65, :])
                        rt = rp.tile([128, 4], F, name="rt")
                        nc.vector.reciprocal(rt[:, :], srt[:, :])
                        nc.sync.dma_start(
                            out=scr[row : row + 1, :].rearrange(
                                "r (p c) -> (r p) c", c=4),
                            in_=rt[:, :],
                        )
                        bct = bcp.tile([64, 512], F, name="bct")
                        src = scr[row : row + 1, :]
                        bap = bass.AP(
                            tensor=src.tensor, offset=src.offset,
                            ap=[[0, 64]] + [list(p) for p in src.ap[1:]],
                        )
                        nc.sync.dma_start(out=bct[:, :], in_=bap)
                        nc.vector.tensor_tensor(
                            Ob[64 * h : 64 * h + 64, i * 512 : (i + 1) * 512],
                            avs[0:64, :], bct[:, :], MULT,
                        )
                    pending.append(i)
                    if filler is not None:
                        filler(i)
                for i2 in pending:
                    y_unit(b, Ob, i2)

            phase_a_alloc(0)
            for b in range(B):
                if b + 1 < B:
                    phase_a_alloc(b + 1)
                    fil = (lambda i, nb=b + 1: phase_a_unit(nb, i))
                else:
                    fil = None
                # batch 0's projection blocks are emitted just-in-time ahead
                # of the attention block that first needs them
                pre = (lambda i: phase_a_unit(0, i)) if b == 0 else None
                phase_d(b, filler=fil, pre=pre)

    _split_sem_waits(nc)
    return nc


# --------------------------------------------------------------------------
def _host_inputs(x, Wq, Wk, Wv):
    """Per-core input dicts (all shared arrays built once)."""
    BF = np.float16
    xT = np.ascontiguousarray(
        np.asarray(x, dtype=np.float32).reshape(BT, C).T).astype(BF)

    # NeoX d-permutation within each head: evens then odds
    dperm = np.concatenate([np.arange(0, D, 2), np.arange(1, D, 2)])

    inv_freq = (1.0 / (10000.0 ** (np.arange(0, D, 2) / D))).astype(np.float64)
    pos = np.arange(T, dtype=np.float64)
    ang = pos[None, :] * inv_freq[:, None]  # (32, T)
    cos32 = np.cos(ang).astype(np.float32)
    sin32 = np.sin(ang).astype(np.float32)
    cos_t = np.tile(np.vstack([cos32, cos32]), (2, 1))  # (128, T)
    sin_t = np.tile(np.vstack([-sin32, sin32]), (2, 1))  # (128, T), sign folded

    p2 = np.zeros((128, 128), dtype=np.float32)
    for hb in (0, 64):
        for i2 in range(32):
            p2[hb + i2, hb + 32 + i2] = 1.0
            p2[hb + 32 + i2, hb + i2] = 1.0

    band = np.where(
        np.arange(128)[None, :] < np.arange(128)[:, None], np.float32(NEG), 0.0
    ).astype(np.float32)
    band2x = np.concatenate([band, band], axis=1)  # (128, 256)
    id2 = np.tile(np.eye(D, dtype=np.float32), (2, 1))  # (128, 64)

    Wq = np.asarray(Wq, dtype=np.float32)
    Wk = np.asarray(Wk, dtype=np.float32)
    Wv = np.asarray(Wv, dtype=np.float32)

    in_maps = []
    for c in range(N_CORES):
        sl = slice(128 * c, 128 * (c + 1))
        wq_c = Wq[:, sl].reshape(C, 2, D)[:, :, dperm].reshape(C, 128)
        wk_c = Wk[:, sl].reshape(C, 2, D)[:, :, dperm].reshape(C, 128)
        in_maps.append({
            "xT": xT,
            "wq": np.ascontiguousarray(wq_c).astype(BF),
            "wk": np.ascontiguousarray(wk_c).astype(BF),
            "wv": np.ascontiguousarray(Wv[:, sl]).astype(BF),
            "wo": None,  # set below
            "cos": cos_t.astype(BF),
            "sin2": sin_t.astype(BF),
            "p2": p2.astype(BF),
            "band2x": band2x,
            "id2": id2,
        })
    return in_maps


def kernel(x, Wq, Wk, Wv, Wo, bo):
    global _BUILT, LAST_RESULT
    from concourse.bass_utils import run_bass_kernel_spmd

    if TRACE:
        _install_ntff_hook()

    if _BUILT is None:
        _BUILT = _build()
    nc = _BUILT

    in_maps = _host_inputs(x, Wq, Wk, Wv)
    Wo = np.asarray(Wo, dtype=np.float32)
    for c in range(N_CORES):
        in_maps[c]["wo"] = np.ascontiguousarray(
            Wo[128 * c : 128 * (c + 1), :]).astype(np.float16)

    last_err = None
    for attempt in range(3):
        try:
            res = run_bass_kernel_spmd(
                nc, in_maps, core_ids=list(range(N_CORES)), trace=TRACE
            )
            break
        except Exception as e:  # transient NRT device errors: retry
            last_err = e
            import time as _time

            _time.sleep(2.0)
    else:
        raise last_err
    LAST_RESULT = res

    acc = res.results[0]["y"].astype(np.float64)
    for c in range(1, N_CORES):
        acc = acc + res.results[c]["y"]
    out = acc.astype(np.float32) + np.asarray(bo, dtype=np.float32)[None, :]
    return out.reshape(B, T, C)
